# revision 1
# baseline (speedup 1.0000x reference)
"""ContrastAwareAttentionBlock Trainium2 Bass kernel.

Sharding: 8 cores = (batch 4) x (image half: rows 0-63 / 64-127); each core
computes its half with a 6-row halo of redundant compute — no collectives.

Layout: channel-major [64ch (+2 indicator rows), pixels], rows padded to 130
cols. All matmuls bf16 (fp32 PSUM accumulation). Conv 3x3 = 9 PSUM-accumulated
matmuls over shifted slices of the same tile; BN folded into weights (scale)
and an indicator-channel matmul term (bias), so out-of-image pixels stay
exactly 0 and one SPMD program serves both halves. ReLU fused into the
ACT-engine PSUM evacuation. Attention per 3-row chunk and patch-position p:
logits matmul -> exp on ACT (with -80 bias so invalid pixels give ~0) -> exp
replicated to (head,dim) channels via 0/1-selector matmuls (two q's at once
into a 128-partition PSUM) -> DVE multiply against stacked v tiles
([v; v>>1col] and [v; v>>1row]) -> q-accumulate -> multiply by reciprocal of
PE-computed replicated softmax denominator (invalid pixels get a huge
denominator via the inverse-indicator channel, flushing them to 0). Fold+
projection fused as 9 shifted PSUM-accumulated matmuls over a 3-chunk xw ring
buffer.
"""
import sys
sys.path.insert(0, '/opt/trn_rl_repo')
import numpy as np

import concourse.bass as bass
import concourse.tile as tile
from concourse import mybir
from concourse.ap import AP
from concourse.bass_utils import run_bass_kernel_spmd

F32 = mybir.dt.float32
F32R = mybir.dt.float32r
BF16 = mybir.dt.bfloat16
U16 = mybir.dt.uint16
AF = mybir.ActivationFunctionType
OP = mybir.AluOpType

B, C, H, W, HEADS, HD = 4, 64, 128, 128, 8, 8
PW = W + 2
TROWS = 78            # tile rows map to image rows [s-7, s+71)
NPX = TROWS * PW
N_CORES = 8
BN_EPS = 1e-5
BIG = 80.0            # exp(-BIG) ~ 0 for invalid pixels
BIGD = 1e30           # denominator for invalid pixels
LARGE = 1e30          # relu clamp for invalid pixels
CONVN = 390           # attention matmul chunk (3 rows, 1 PSUM bank)
CVN = 512             # conv matmul chunk (full PSUM bank)
MAX_WAITS = 1

TAPS = [(di, dj) for di in (-1, 0, 1) for dj in (-1, 0, 1)]


def _split_excess_waits(nc):
    """This walrus build rejects >1 sync wait per instruction; move excess
    waits onto same-engine NOPs inserted before the offender."""
    bbs, fixups = [], {}
    for f in nc.m.functions:
        for bb in f.blocks:
            bbs.append(bb)
            for inst in bb.instructions:
                si = inst.sync_info
                waits = list(si.on_wait) if si is not None and si.on_wait else []
                if len(waits) > MAX_WAITS:
                    si.on_wait = waits[:MAX_WAITS]
                    rest = waits[MAX_WAITS:]
                    chunks = [rest[i:i + MAX_WAITS]
                              for i in range(0, len(rest), MAX_WAITS)]
                    fixups.setdefault(id(bb), {}).setdefault(inst.name, []).extend(
                        (inst.engine, ch) for ch in chunks)
    if not fixups:
        return
    created = {}
    for bb in bbs:
        for name, specs in fixups.get(id(bb), {}).items():
            nops = []
            for engine, ch in specs:
                bi = nc.engines[engine].nop(nofuse=True)
                bi.ins.sync_info = mybir.SyncInfo(on_wait=ch, on_update=[])
                nops.append(bi.ins)
            created[name] = nops
    all_nops = {n.name for ns in created.values() for n in ns}
    for bb in bbs:
        insts = [i for i in bb.instructions if i.name not in all_nops]
        new = []
        for inst in insts:
            new.extend(created.get(inst.name, ()))
            new.append(inst)
        bb.instructions = new


class _TCtx(tile.TileContext):
    def _drain_and_barrier(self, tick_clock, wait_clock):
        from concourse.tile import ScopedClock
        probe = self.nc.sync.nop(nofuse=True)
        wait_clock.add_sem_waits(
            probe.ins, ScopedClock({None: tick_clock.global_clock}))
        self.nc.sync.drain()
        self.nc.all_engine_barrier()
        assert self.sems is not None
        popped = self.nc._tile_sem_poison_stack.pop()
        assert popped is self._sem_poison
        self.nc.clear_and_free_semaphores(list(self.sems.allocated().values()))
        self.nc.all_engine_barrier()

    def __exit__(self, exc_type, exc_val, exc_tb):
        ret = super().__exit__(exc_type, exc_val, exc_tb)
        if exc_type is None:
            _split_excess_waits(self.nc)
        return ret


def _prep_weights(conv_w, bn_g, bn_b, bn_m, bn_v, w_v, b_v, w_attn, b_attn,
                  w_proj, b_proj):
    sc = HD ** -0.5
    inv = bn_g / np.sqrt(bn_v + BN_EPS)          # [4, 64]
    beta = bn_b - bn_m * inv                     # [4, 64]
    cols, parts = {}, []

    def add(name, a):
        a = np.asarray(a, np.float32)
        full = np.zeros((72, a.shape[1]), np.float32)
        full[:a.shape[0]] = a
        cols[name] = sum(p.shape[1] for p in parts)
        parts.append(full)

    # conv taps: pairs {(0,dj),(-1,dj)} contract 128 over the row-stacked
    # input ([x; x shifted one row down]); singles (1,dj) contract 64;
    # beta + the -LARGE out-of-image clamp ride a 2-row indicator matmul.
    def wtap(k, di, dj):
        return conv_w[k, :, :, di + 1, dj + 1].T * inv[k][None, :]

    csing = np.zeros((64, 4 * 3 * 64), np.float32)
    cbias = np.zeros((2, 4 * 64), np.float32)
    for k in range(4):
        for j, dj in enumerate((-1, 0, 1)):
            csing[:, (k * 3 + j) * 64:(k * 3 + j) * 64 + 64] = wtap(k, 1, dj)
        cbias[0, k * 64:k * 64 + 64] = beta[k]
        cbias[1, k * 64:k * 64 + 64] = -LARGE
    add("csing", csing)
    add("cbias", cbias)
    wv65 = np.zeros((65, 64), np.float32)
    wv65[:64] = w_v.T
    wv65[64] = b_v
    add("wv", wv65)
    # attention logits, packed 128-out: channel layout per p (80 channels,
    # 16-aligned blocks so selector windows never straddle tiles):
    #   +0: q0(8n) q1(8n) | +16: q3 q4 | +32: q6 q7 | +48: q2 q5 | +64: q8
    #   +72..79: zero pad.  6 tiles of 128 (wa6 padded to 768 cols; the
    #   pad channels get logits 0 -> exp 1, killed by zero bcast weights).
    # bias on indicator row, -BIG on inverse-indicator row.
    QBLK = [(0, (0, 1)), (16, (3, 4)), (32, (6, 7)), (48, (2, 5)),
            (64, (8,))]
    wa6 = np.zeros((66, 768), np.float32)
    for p in range(9):
        for bo, qs in QBLK:
            for j, q in enumerate(qs):
                for n in range(HEADS):
                    row = n * 81 + p * 9 + q
                    ch = 80 * p + bo + j * 8 + n
                    wa6[:64, ch] = w_attn[row] * sc
                    wa6[64, ch] = b_attn[row] * sc
                    wa6[65, ch] = -BIG
    add("wa6", wa6)
    # bcastT: per tile i, [72 -> 128] replicating compact rden (p,n) into
    # the tile's channel layout (zero for pad channels).
    bct = np.zeros((72, 6 * 128), np.float32)
    for g in range(720):
        p, off = divmod(g, 80)
        if off < 72:
            n = off % 8
            bct[p * 8 + n, (g // 128) * 128 + (g % 128)] = 1.0
    add("bct", bct)
    bigd72 = np.zeros((66, 72), np.float32)
    bigd72[65] = BIGD                            # inverse-indicator row
    add("bigd72", bigd72)
    idm = np.zeros((72, 64), np.float32)
    idm[:64] = np.eye(64, dtype=np.float32)
    add("id64", idm)
    add("wp", w_proj.T)
    bpi = np.zeros((66, 64), np.float32)
    bpi[64] = b_proj                             # indicator row
    bpi[65] = -LARGE
    add("bpi", bpi)
    bpineg = np.zeros((66, 64), np.float32)
    bpineg[64] = -b_proj
    bpineg[65] = -LARGE
    add("bpineg", bpineg)

    # ---- 128-partition weights (wts2) ----
    cols2, parts2 = {}, []

    def add2(name, a):
        a = np.asarray(a, np.float32)
        full = np.zeros((128, a.shape[1]), np.float32)
        full[:a.shape[0]] = a
        cols2[name] = sum(p.shape[1] for p in parts2)
        parts2.append(full)

    # fold128: [128, 64], sums upper+lower 64-halves
    fold = np.zeros((128, 64), np.float32)
    fold[:64] = np.eye(64, dtype=np.float32)
    fold[64:] = np.eye(64, dtype=np.float32)
    add2("fold", fold)
    cpair = np.zeros((128, 4 * 3 * 64), np.float32)
    for k in range(4):
        for j, dj in enumerate((-1, 0, 1)):
            cpair[0:64, (k * 3 + j) * 64:(k * 3 + j) * 64 + 64] = \
                wtap(k, 0, dj)
            cpair[64:128, (k * 3 + j) * 64:(k * 3 + j) * 64 + 64] = \
                wtap(k, -1, dj)
    add2("cpair", cpair)
    # den6: per tile i, [128 -> 72] compact softmax denominator (p,n)
    den6 = np.zeros((128, 6 * 72), np.float32)
    for g in range(720):
        p, off = divmod(g, 80)
        if off < 72:
            n = off % 8
            den6[g % 128, (g // 128) * 72 + p * 8 + n] = 1.0
    add2("den6", den6)
    # selw: per (p,k): [32, 128] block at 32-aligned partition window w32;
    # rows w32+s.. map the A-tile window channels to (n,d)[-pair] outputs.
    selw = np.zeros((128, 45 * 128), np.float32)
    for p in range(9):
        for k, (bo, qs) in enumerate(QBLK):
            g0 = 80 * p + bo
            o = g0 % 128
            w0 = 64 if o >= 64 else o - o % 32
            col = (p * 5 + k) * 128
            for n in range(HEADS):
                for d in range(HD):
                    selw[o + n, col + n * 8 + d] = 1.0
                    if len(qs) == 2:
                        selw[o + 8 + n, col + 64 + n * 8 + d] = 1.0
    add2("selw", selw)
    return (np.concatenate(parts, axis=1), cols,
            np.concatenate(parts2, axis=1), cols2)


def _build(wcols, wtotal, wcols2, wtotal2):
    nc = bass.Bass("TRN2", target_bir_lowering=False, debug=False)
    xs = nc.dram_tensor("xs", [66, NPX], F32, kind="ExternalInput").ap()
    wts = nc.dram_tensor("wts", [72, wtotal], F32, kind="ExternalInput").ap()
    wts2 = nc.dram_tensor("wts2", [128, wtotal2], F32,
                          kind="ExternalInput").ap()
    y = nc.dram_tensor("y", [C, 64 * W], F32, kind="ExternalOutput").ap()

    def wr_at(w, name, r0, r1, n=64):
        return w[r0:r1, wcols[name]:wcols[name] + n]

    with _TCtx(nc) as tc:
        cpool = tc.alloc_tile_pool(name="const", bufs=1)
        spool = tc.alloc_tile_pool(name="stage", bufs=2)
        rpool_ = tc.alloc_tile_pool(name="raw", bufs=1, side="right")

        wf = rpool_.tile([72, wtotal], F32, tag="wf")
        nc.sync.dma_start(wf[:], wts[:])
        wr = cpool.tile([72, wtotal], BF16)
        nc.vector.tensor_copy(wr[:], wf[:])
        w2f = rpool_.tile([128, wtotal2], F32, tag="w2f")
        nc.sync.dma_start(w2f[:], wts2[:])
        w2 = cpool.tile([128, wtotal2], BF16)
        nc.vector.tensor_copy(w2[:], w2f[:])
        fold128 = w2[:, wcols2["fold"]:wcols2["fold"] + 64]
        wb = cpool.tile([72, 192], BF16)
        nc.vector.tensor_copy(wb[:, 0:64], wf[:, wcols["wp"]:wcols["wp"] + 64])
        nc.vector.tensor_copy(wb[:, 64:128],
                              wf[:, wcols["id64"]:wcols["id64"] + 64])
        nc.vector.tensor_scalar_mul(wb[:, 128:192],
                                    wf[:, wcols["wp"]:wcols["wp"] + 64], -1.0)

        ind2 = cpool.tile([2, NPX], BF16)

        def conv_matmuls(ps, src, k, o, n):
            first = True
            for j in range(3):
                cp = wcols2["cpair"] + (k * 3 + j) * 64
                nc.tensor.matmul(ps[:, :n], w2[0:128, cp:cp + 64],
                                 src[0:128, o + j - 1:o + j - 1 + n],
                                 start=first, stop=False)
                first = False
            for j in range(3):
                cs = wcols["csing"] + (k * 3 + j) * 64
                nc.tensor.matmul(ps[:, :n], wr[0:64, cs:cs + 64],
                                 src[0:64, o + PW + j - 1:o + PW + j - 1 + n],
                                 start=False, stop=False)
            cb = wcols["cbias"] + k * 64
            nc.tensor.matmul(ps[:, :n], wr[0:2, cb:cb + 64],
                             ind2[0:2, o:o + n], start=False, stop=True)

        def conv_stage(cps, src, dst, k, r0, r1, stack):
            lo, hi = r0 * PW + 1, r1 * PW - 1
            for o in range(lo, hi, CVN):
                n = min(CVN, hi - o)
                ps = cps.tile([64, CVN], F32)
                conv_matmuls(ps, src, k, o, n)
                nc.scalar.activation(dst[0:64, o:o + n], ps[:, :n], AF.Relu)
                if stack:
                    nc.sync.dma_start(dst[64:128, o + PW:o + PW + n],
                                      dst[0:64, o:o + n])
            # interior padding cols come out exactly 0 from the -LARGE
            # indicator clamp + ReLU; only the two corner elements outside
            # the evacuation span [lo, hi) need explicit zeroing.
            du = dst[0:64].bitcast(U16)
            nc.gpsimd.memset(du[:, r0 * PW:r0 * PW + 1], 0.0)
            nc.gpsimd.memset(du[:, r1 * PW - 1:r1 * PW], 0.0)
            if stack:
                dl = dst[64:128].bitcast(U16)
                nc.gpsimd.memset(dl[:, (r0 + 1) * PW:(r0 + 1) * PW + 1], 0.0)
                nc.gpsimd.memset(dl[:, (r1 + 1) * PW - 1:(r1 + 1) * PW], 0.0)

        # ---- input & convs 1,2 ----
        x0f = rpool_.tile([66, NPX], F32, tag="x0f")
        nc.sync.dma_start(x0f[:], xs[:])
        x0s = spool.tile([128, NPX], BF16, tag="sbig")
        nc.vector.tensor_copy(ind2[:], x0f[64:66, :])
        NQ = NPX // 4
        for j in range(4):
            a, b = j * NQ, NPX if j == 3 else (j + 1) * NQ
            nc.vector.tensor_copy(x0s[0:64, a:b], x0f[0:64, a:b])
            nc.sync.dma_start(x0s[64:128, a + PW:b + PW] if j < 3 else
                              x0s[64:128, a + PW:NPX],
                              x0s[0:64, a:b] if j < 3 else
                              x0s[0:64, a:NPX - PW])
        rpool_.release()
        x1s = spool.tile([128, NPX], BF16, tag="sbig")
        x2 = spool.tile([66, NPX], BF16, tag="stage", bufs=1)
        nc.sync.dma_start(x2[64:66, :], ind2[0:2, :])
        with tc.tile_pool(name="cps", bufs=4, space="PSUM") as cps:
            conv_stage(cps, x0s, x1s, 0, 2, 76, stack=True)
            conv_stage(cps, x1s, x2, 1, 3, 75, stack=False)

        # ---- attention ----
        z = spool.tile([128, NPX], BF16, tag="sbig")

        GR0, GR1 = 4, 74
        ZR0, ZR1 = 5, 73
        n_chunks = (GR1 - GR0 + 2) // 3
        QBLK_BO = [0, 16, 32, 48, 64]

        vpool = tc.alloc_tile_pool(name="vb", bufs=2)
        epool = tc.alloc_tile_pool(name="ep", bufs=2)
        rpool = tc.alloc_tile_pool(name="rd", bufs=2)
        tpool = tc.alloc_tile_pool(name="tmp", bufs=2)
        xwpool = tc.alloc_tile_pool(name="xw", bufs=3)
        mps = tc.alloc_tile_pool(name="mps", bufs=2, space="PSUM")
        vy = tc.alloc_tile_pool(name="vy", bufs=1, space="PSUM")
        apool = tc.alloc_tile_pool(name="aps", bufs=2, space="PSUM")
        fps = tc.alloc_tile_pool(name="fps", bufs=2, space="PSUM")

        xw_tiles = {}

        def chunk_rows(ci):
            t0 = GR0 + 3 * ci
            return t0, min(t0 + 3, GR1)

        def get_xw(p, ci):
            # xw tile for chunk ci covers rows [t0-1, t1+1) (5*PW cols);
            # the halo rows are written by the neighbor chunks' products.
            if (p, ci) in xw_tiles:
                return xw_tiles[(p, ci)]
            t0, t1 = chunk_rows(ci)
            xw = xwpool.tile([64, 5 * PW], BF16, tag=f"xw{p}")
            xwf = xw[:].bitcast(U16)
            # zero the padding columns the evacuation regions never write
            nc.gpsimd.memset(xwf[:, 0:1], 0.0)
            nc.gpsimd.memset(xwf[:, PW - 1:PW + 1], 0.0)
            nc.gpsimd.memset(xwf[:, 4 * PW - 1:4 * PW + 1], 0.0)
            nc.gpsimd.memset(xwf[:, 5 * PW - 1:5 * PW], 0.0)
            if ci == 0:
                nc.gpsimd.memset(xwf[:, 0:PW + 1], 0.0)
            xw_tiles[(p, ci)] = (xw, t0)
            return xw_tiles[(p, ci)]

        def do_products(ci):
            t0, t1 = chunk_rows(ci)
            S0 = t0 * PW + 1
            SN = (t1 - t0) * PW - 2
            vbase = (t0 - 1) * PW
            vlen = (t1 - t0 + 2) * PW
            vb = vpool.tile([128, 5 * PW], BF16, tag="vb")
            vbr = vpool.tile([128, 5 * PW], BF16, tag="vbr")
            for j in range(2):
                vo = j * 390
                vn = min(390, vlen - vo)
                if vn <= 0:
                    continue
                psv = vy.tile([64, 390], F32, tag="v")
                nc.tensor.matmul(psv[:, :vn], wr_at(wr, "wv", 0, 65),
                                 x2[0:65, vbase + vo:vbase + vo + vn],
                                 start=True, stop=True)
                nc.scalar.copy(vb[0:64, vo:vo + vn], psv[:, :vn])
                if vo == 0:
                    nc.scalar.copy(vb[64:128, 0:vn - 1], psv[:, 1:vn])
                    nc.scalar.copy(vbr[0:64, 0:vn], psv[:, :vn])
                    nc.scalar.copy(vbr[64:128, 0:vn - PW], psv[:, PW:vn])
                else:
                    nc.scalar.copy(vb[64:128, vo - 1:vo - 1 + vn], psv[:, :vn])
                    nc.scalar.copy(vbr[0:64, vo:vo + vn], psv[:, :vn])
                    nc.scalar.copy(vbr[64:128, vo - PW:vo - PW + vn],
                                   psv[:, :vn])
            # packed logits -> exp
            E = []
            for i in range(6):
                psl = mps.tile([128, CONVN], F32, tag="m")
                nc.tensor.matmul(psl[:, :SN],
                                 wr[0:66, wcols["wa6"] + i * 128:
                                    wcols["wa6"] + i * 128 + 128],
                                 x2[0:66, S0:S0 + SN], start=True, stop=True)
                e = epool.tile([128, CONVN], BF16, tag=f"E{i}")
                nc.scalar.activation(e[:, :SN], psl[:, :SN], AF.Exp)
                E.append(e)
            # compact denominator (p,n) + reciprocal
            psd = mps.tile([128, CONVN], F32, tag="m")
            for i in range(6):
                nc.tensor.matmul(psd[0:72, :SN],
                                 w2[0:128, wcols2["den6"] + i * 72:
                                    wcols2["den6"] + i * 72 + 72],
                                 E[i][:, :SN], start=(i == 0), stop=False)
            nc.tensor.matmul(psd[0:72, :SN], wr_at(wr, "bigd72", 64, 66, 72),
                             x2[64:66, S0:S0 + SN], start=False, stop=True)
            rdc = rpool.tile([72, CONVN], F32, tag="rdc")
            nc.vector.reciprocal(rdc[:, :SN], psd[0:72, :SN])
            rdcb = rpool.tile([72, CONVN], BF16, tag="rdcb")
            nc.gpsimd.tensor_copy(rdcb[:, :SN], rdc[:, :SN])
            # broadcast rden into tile layout, normalize exp -> A tiles
            A = []
            for i in range(6):
                psr = mps.tile([128, CONVN], F32, tag="m")
                nc.tensor.matmul(psr[:, :SN],
                                 wr[0:72, wcols["bct"] + i * 128:
                                    wcols["bct"] + i * 128 + 128],
                                 rdcb[:, :SN], start=True, stop=True)
                rt = epool.tile([128, CONVN], BF16, tag=f"R{i}")
                nc.scalar.copy(rt[:, :SN], psr[:, :SN])
                a = epool.tile([128, CONVN], BF16, tag=f"A{i}")
                nc.vector.tensor_tensor(a[:, :SN], E[i][:, :SN], rt[:, :SN],
                                        OP.mult)
                A.append(a)
            # products + fold, per patch position p
            SVB = S0 - vbase
            VOFF = [(0, SVB - PW - 1), (0, SVB - 1), (0, SVB + PW - 1),
                    (1, SVB - PW + 1), (2, SVB + PW + 1)]
            for p in range(9):
                tms = []
                for k in range(5):
                    g0 = 80 * p + QBLK_BO[k]
                    i, o = divmod(g0, 128)
                    w0 = 64 if o >= 64 else o - o % 32
                    ww = 64 if o >= 64 else 32
                    wid = 128 if k < 4 else 64
                    scol = wcols2["selw"] + (p * 5 + k) * 128
                    psp = apool.tile([128, CONVN], F32, tag="a")
                    nc.tensor.matmul(psp[0:wid, :SN],
                                     w2[w0:w0 + ww, scol:scol + wid],
                                     A[i][w0:w0 + ww, :SN],
                                     start=True, stop=True)
                    which, vlo = VOFF[k]
                    vt = (vb, vbr, vb)[which]
                    tm = tpool.tile([128, CONVN], BF16, tag=f"tm{k}")
                    nc.vector.tensor_tensor(tm[0:wid, :SN], psp[0:wid, :SN],
                                            vt[0:wid, vlo:vlo + SN], OP.mult)
                    tms.append(tm)
                psf = fps.tile([64, CONVN], F32, tag="f")
                for k in range(4):
                    nc.tensor.matmul(psf[:, :SN], fold128[:, :],
                                     tms[k][:, :SN], start=(k == 0),
                                     stop=False)
                nc.tensor.matmul(psf[:, :SN], wb[0:64, 64:128],
                                 tms[4][0:64, :SN], start=False, stop=True)
                # evacuate: middle rows into this chunk's xw tile, first/last
                # rows into the neighbors' halo rows.
                xw, _ = get_xw(p, ci)
                nc.scalar.copy(xw[:, PW + 1:PW + 1 + SN], psf[:, :SN])
                if ci > 0:
                    xwp, _ = xw_tiles[(p, ci - 1)]
                    nc.gpsimd.tensor_copy(xwp[:, 4 * PW + 1:5 * PW - 1],
                                          xw[:, PW + 1:2 * PW - 1])
                if ci + 1 < n_chunks:
                    xwn, _ = get_xw(p, ci + 1)
                    nc.gpsimd.tensor_copy(xwn[:, 1:PW - 1],
                                          xw[:, 3 * PW + 1:4 * PW - 1])

        def do_proj(ci):
            t0, t1 = chunk_rows(ci)
            zr0, zr1 = max(t0, ZR0), min(t1, ZR1)
            if zr0 >= zr1:
                return
            zS0 = zr0 * PW + 1
            zSN = (zr1 - zr0) * PW - 2
            base = (t0 - 1) * PW

            def accum(psy, wcol, bias_name):
                for p, (di, dj) in enumerate(TAPS):
                    off = di * PW + dj
                    xw, _ = xw_tiles[(p, ci)]
                    la = zS0 - off - base
                    nc.tensor.matmul(psy[:, :zSN], wb[0:64, wcol:wcol + 64],
                                     xw[0:64, la:la + zSN],
                                     start=(p == 0), stop=False)
                nc.tensor.matmul(psy[:, :zSN], wr_at(wr, bias_name, 64, 66),
                                 x2[64:66, zS0:zS0 + zSN],
                                 start=False, stop=True)

            psy = vy.tile([64, 390], F32, tag="y")
            accum(psy, 0, "bpi")
            # rows [ZR0, 7) and [71, ZR1) may be out-of-image (which side
            # depends on the half); those z pixels pick up proj taps from
            # valid xw neighbors, so clamp them to 0 via
            # relu(psy - L*invind) - relu(-psy - L*invind).
            edge = [(r0, r1) for (r0, r1) in ((zr0, min(zr1, 7)),
                                              (max(zr0, 71), zr1))
                    if r0 < r1]
            if edge:
                psyn = mps.tile([64, 390], F32, tag="m")
                accum(psyn, 128, "bpineg")
                for (r0, r1) in edge:
                    e0 = r0 * PW + 1 - zS0
                    en = (r1 - r0) * PW - 2
                    tp = tpool.tile([64, 390], F32, tag="zp", bufs=1)
                    nc.scalar.activation(tp[:, :en], psy[:, e0:e0 + en],
                                         AF.Relu)
                    tn = tpool.tile([64, 390], F32, tag="zn", bufs=1)
                    nc.scalar.activation(tn[:, :en], psyn[:, e0:e0 + en],
                                         AF.Relu)
                    nc.vector.tensor_tensor(
                        z[0:64, r0 * PW + 1:r0 * PW + 1 + en],
                        tp[:, :en], tn[:, :en], OP.subtract)
            m0, m1 = max(zr0, 7), min(zr1, 71)
            if m0 < m1:
                e0 = m0 * PW + 1 - zS0
                en = (m1 - m0) * PW - 2
                nc.scalar.copy(z[0:64, m0 * PW + 1:m0 * PW + 1 + en],
                               psy[:, e0:e0 + en])
            # mirror the written rows into the stacked lower half
            nc.sync.dma_start(z[64:128, zS0 + PW:zS0 + PW + zSN],
                              z[0:64, zS0:zS0 + zSN])

        for ci in range(n_chunks):
            do_products(ci)
            if ci >= 1:
                do_proj(ci - 1)
        do_proj(n_chunks - 1)
        z3 = z[0:64].bitcast(U16).rearrange("c (r w) -> c r w", r=TROWS, w=PW)
        nc.gpsimd.memset(z3[:, ZR0:ZR1, 0:1], 0.0)
        nc.gpsimd.memset(z3[:, ZR0:ZR1, PW - 1:PW], 0.0)
        z3l = z[64:128].bitcast(U16).rearrange("c (r w) -> c r w",
                                               r=TROWS, w=PW)
        nc.gpsimd.memset(z3l[:, ZR0 + 1:ZR1 + 1, 0:1], 0.0)
        nc.gpsimd.memset(z3l[:, ZR0 + 1:ZR1 + 1, PW - 1:PW], 0.0)

        for pool in (fps, apool, vy, mps, xwpool, tpool, rpool,
                     epool, vpool):
            pool.release()

        # ---- convs 3,4 and output ----
        y3s = spool.tile([128, NPX], BF16, tag="sbig")
        ypool = tc.alloc_tile_pool(name="yb", bufs=2)
        with tc.tile_pool(name="cps2", bufs=4, space="PSUM") as cps2:
            conv_stage(cps2, z, y3s, 2, 6, 72, stack=True)
            # conv4: 3-image-row windows evacuated straight to the output
            y3d = y[:].rearrange("c (r w) -> c r w", r=64, w=W)
            for r in range(7, 71, 3):
                nr = min(3, 71 - r)
                o = r * PW + 1
                n = nr * PW - 2
                ps = cps2.tile([64, CVN], F32)
                conv_matmuls(ps, y3s, 3, o, n)
                yb = ypool.tile([64, 390], F32, tag="yb")
                nc.scalar.activation(yb[:, :n], ps[:, :n], AF.Relu)
                ybv = AP(yb.tensor, yb.offset,
                         [[390, 64], [PW, nr], [1, W]])
                nc.sync.dma_start(y3d[:, r - 7:r - 7 + nr, :], ybv)
        ypool.release()
        spool.release()
        cpool.release()
    return nc


_CACHE = {}
TRACE_DIR = None
LAST_EXEC_NS = None


def kernel(**inputs):
    x = np.asarray(inputs["x"], np.float32)
    conv_w = np.asarray(inputs["conv_w"], np.float32)
    args = (conv_w, np.asarray(inputs["bn_g"], np.float32),
            np.asarray(inputs["bn_b"], np.float32),
            np.asarray(inputs["bn_m"], np.float32),
            np.asarray(inputs["bn_v"], np.float32),
            np.asarray(inputs["w_v"], np.float32),
            np.asarray(inputs["b_v"], np.float32),
            np.asarray(inputs["w_attn"], np.float32),
            np.asarray(inputs["b_attn"], np.float32),
            np.asarray(inputs["w_proj"], np.float32),
            np.asarray(inputs["b_proj"], np.float32))
    wts, wcols, wts2, wcols2 = _prep_weights(*args)

    # per-core input shards with indicator channels
    shards = []
    for core in range(N_CORES):
        bb, half = divmod(core, 2)
        s = half * 64
        rowbase = s - 7
        sh = np.zeros((66, TROWS, PW), np.float32)
        r0, r1 = max(0, rowbase + 1), min(H, rowbase + 77)
        sh[:64, r0 - rowbase:r1 - rowbase, 1:1 + W] = x[bb, :, r0:r1, :]
        sh[64, r0 - rowbase:r1 - rowbase, 1:1 + W] = 1.0
        sh[65] = 1.0 - sh[64]
        shards.append(sh.reshape(66, NPX))

    key = "k1"
    if key not in _CACHE:
        _CACHE[key] = _build(wcols, wts.shape[1], wcols2, wts2.shape[1])
    nc = _CACHE[key]
    in_maps = [{"xs": shards[i], "wts": wts, "wts2": wts2}
               for i in range(N_CORES)]
    kw = {}
    if TRACE_DIR is not None:
        kw = dict(trace=True, tmpdir=TRACE_DIR)
    res = run_bass_kernel_spmd(nc, in_maps, core_ids=list(range(N_CORES)), **kw)
    global LAST_EXEC_NS
    LAST_EXEC_NS = res.exec_time_ns
    return out_from(res)


def out_from(res):
    out = np.zeros((B, C, H, W), np.float32)
    for core in range(N_CORES):
        bb, half = divmod(core, 2)
        s = half * 64
        out[bb, :, s:s + 64, :] = res.results[core]["y"].reshape(C, 64, W)
    return out



# revision 7
# speedup vs baseline: 1.2522x; 1.2522x over previous
"""ContrastAwareAttentionBlock Trainium2 Bass kernel (s-space restructure).

Sharding: 8 cores = (batch 4) x (image half: rows 0-63 / 64-127); each core
computes its half with a 6-row halo of redundant compute — no collectives.

Attention is computed in "s-space": with s = off_q - off_p (25 values in a
5x5 grid), fold+proj collapse to  z[:,l] = w_proj @ sum_s G_s[n,l] *
v[:, l+s] + b_proj  where  G_s[n,l] = sum_p A[n, l-off_p, p, off_p+s]  and
A is the normalized attention field (packed (p,q,n) channels, 6 tiles of
128).  Per 3-row chunk this needs only 18 G-build matmuls (0/1 selectors
over row-shifted A tiles, accumulated in 2 PSUM tiles of 200 (s,n)
channels), 13 sel expansions ((s,n) -> (s,n,d)), 26 half-DVE multiplies
against a persistent mirrored v field, 13 fold matmuls and a single local
projection — vs 45 sel + 45 fold + 10 shifted-proj matmuls in the direct
(p,q) form.  Invalid pixels die via indicator channels: logits get -BIG,
the softmax denominator gets a huge sentinel channel (exp(ln BIGD) on the
inverse indicator), so A ~ 0 outside the image and all shifted reads are
harmless.  Convs are unchanged: 3x3 conv = 9 PSUM-accumulated matmuls over
shifted slices with BN folded into weights and an indicator-channel bias.
"""
import sys
sys.path.insert(0, '/opt/trn_rl_repo')
import numpy as np

import concourse.bass as bass
import concourse.tile as tile
from concourse import mybir
from concourse.ap import AP
from concourse.bass_utils import run_bass_kernel_spmd

F32 = mybir.dt.float32
BF16 = mybir.dt.bfloat16
U16 = mybir.dt.uint16
AF = mybir.ActivationFunctionType
OP = mybir.AluOpType

B, C, H, W, HEADS, HD = 4, 64, 128, 128, 8, 8
PW = W + 2
TROWS = 78            # tile rows map to image rows [s-7, s+71)
NPX = TROWS * PW
N_CORES = 8
BN_EPS = 1e-5
BIG = 80.0            # exp(-BIG) ~ 0 for invalid pixels
LNBIGD = 69.0         # exp(+LNBIGD) ~ 1e30 denominator for invalid pixels
LARGE = 1e30          # relu clamp for invalid pixels
CONVN = 390           # attention matmul chunk (3 rows, 1 PSUM bank)
CVN = 512             # conv matmul chunk (full PSUM bank)
MAX_WAITS = 1
sc = HD ** -0.5

TAPS = [(di, dj) for di in (-1, 0, 1) for dj in (-1, 0, 1)]

# packed-E layout: 6 tiles x 128; p0..p5 whole, p6,p7,p8 split
P_SEGS = {
    0: [(0, 0, 0, 72)],
    1: [(1, 16, 0, 72)],
    2: [(2, 32, 0, 72)],
    3: [(3, 48, 0, 72)],
    4: [(4, 0, 0, 72)],
    5: [(5, 0, 0, 72)],
    6: [(0, 72, 0, 56), (1, 0, 56, 72)],
    7: [(1, 88, 0, 40), (2, 0, 40, 72)],
    8: [(2, 104, 0, 24), (3, 0, 24, 72)],
}


def _packed_pos(p, q, n):
    ch = 8 * q + n
    for (t, r0, lo, hi) in P_SEGS[p]:
        if lo <= ch < hi:
            return t, r0 + (ch - lo)
    raise AssertionError


def _s_idx(p, q):
    si = TAPS[q][0] - TAPS[p][0]
    sj = TAPS[q][1] - TAPS[p][1]
    return (si + 2) * 5 + (sj + 2)


S_OFF = [((i // 5) - 2, (i % 5) - 2) for i in range(25)]
SEL_GROUPS = [(0, 1), (2, 3), (4, 5), (6, 7), (8, 9), (10, 11), (12, 13),
              (14, 15), (16, 17), (18, 19), (20, 21), (22, 23), (24,)]
# sel matmul windows: (src 0=G0b/1=G1b, base partition in {0,32,64}, size);
# matmul base partitions are restricted to 0/32/64 and lhsT must match.
SEL_WIN = [(0, 0, 32), (0, 0, 32), (0, 32, 32), (0, 32, 32),
           (0, 64, 32), (0, 64, 32), (0, 64, 64), (0, 64, 64),
           (1, 0, 32), (1, 0, 32), (1, 32, 32), (1, 32, 32), (1, 64, 8)]
# G-build segments: (p, tile, has_g0, has_g1); built to match P_SEGS
GB_SEGS = []
for _p in range(9):
    for (_t, _r0, _lo, _hi) in P_SEGS[_p]:
        _g0 = any(8 * _s_idx(_p, ch // 8) + ch % 8 < 128
                  for ch in range(_lo, _hi))
        _g1 = any(8 * _s_idx(_p, ch // 8) + ch % 8 >= 128
                  for ch in range(_lo, _hi))
        GB_SEGS.append((_p, _t, _g0, _g1))


def _split_excess_waits(nc):
    """This walrus build rejects >1 sync wait per instruction; move excess
    waits onto same-engine NOPs inserted before the offender."""
    bbs, fixups = [], {}
    for f in nc.m.functions:
        for bb in f.blocks:
            bbs.append(bb)
            for inst in bb.instructions:
                si = inst.sync_info
                waits = list(si.on_wait) if si is not None and si.on_wait else []
                if len(waits) > MAX_WAITS:
                    si.on_wait = waits[:MAX_WAITS]
                    rest = waits[MAX_WAITS:]
                    chunks = [rest[i:i + MAX_WAITS]
                              for i in range(0, len(rest), MAX_WAITS)]
                    fixups.setdefault(id(bb), {}).setdefault(inst.name, []).extend(
                        (inst.engine, ch) for ch in chunks)
    if not fixups:
        return
    created = {}
    for bb in bbs:
        for name, specs in fixups.get(id(bb), {}).items():
            nops = []
            for engine, ch in specs:
                bi = nc.engines[engine].nop(nofuse=True)
                bi.ins.sync_info = mybir.SyncInfo(on_wait=ch, on_update=[])
                nops.append(bi.ins)
            created[name] = nops
    all_nops = {n.name for ns in created.values() for n in ns}
    for bb in bbs:
        insts = [i for i in bb.instructions if i.name not in all_nops]
        new = []
        for inst in insts:
            new.extend(created.get(inst.name, ()))
            new.append(inst)
        bb.instructions = new


class _TCtx(tile.TileContext):
    def _drain_and_barrier(self, tick_clock, wait_clock):
        from concourse.tile import ScopedClock
        probe = self.nc.sync.nop(nofuse=True)
        wait_clock.add_sem_waits(
            probe.ins, ScopedClock({None: tick_clock.global_clock}))
        self.nc.sync.drain()
        self.nc.all_engine_barrier()
        assert self.sems is not None
        popped = self.nc._tile_sem_poison_stack.pop()
        assert popped is self._sem_poison
        self.nc.clear_and_free_semaphores(list(self.sems.allocated().values()))
        self.nc.all_engine_barrier()

    def __exit__(self, exc_type, exc_val, exc_tb):
        ret = super().__exit__(exc_type, exc_val, exc_tb)
        if exc_type is None:
            _split_excess_waits(self.nc)
        return ret


def _prep_weights(conv_w, bn_g, bn_b, bn_m, bn_v, w_v, b_v, w_attn, b_attn,
                  w_proj, b_proj):
    inv = bn_g / np.sqrt(bn_v + BN_EPS)          # [4, 64]
    beta = bn_b - bn_m * inv                     # [4, 64]
    cols, parts = {}, []

    def add(name, a):
        a = np.asarray(a, np.float32)
        full = np.zeros((72, a.shape[1]), np.float32)
        full[:a.shape[0]] = a
        cols[name] = sum(p.shape[1] for p in parts)
        parts.append(full)

    # conv taps: pairs {(0,dj),(-1,dj)} contract 128 over the row-stacked
    # input ([x; x shifted one row down]); singles (1,dj) contract 64;
    # beta + the -LARGE out-of-image clamp ride a 2-row indicator matmul.
    def wtap(k, di, dj):
        return conv_w[k, :, :, di + 1, dj + 1].T * inv[k][None, :]

    csing = np.zeros((64, 4 * 3 * 64), np.float32)
    cbias = np.zeros((2, 4 * 64), np.float32)
    for k in range(4):
        for j, dj in enumerate((-1, 0, 1)):
            csing[:, (k * 3 + j) * 64:(k * 3 + j) * 64 + 64] = wtap(k, 1, dj)
        cbias[0, k * 64:k * 64 + 64] = beta[k]
        cbias[1, k * 64:k * 64 + 64] = -LARGE
    add("csing", csing)
    add("cbias", cbias)
    wv65 = np.zeros((65, 64), np.float32)
    wv65[:64] = w_v.T
    wv65[64] = b_v
    add("wv", wv65)
    # packed attention logits [66, 6*128]: channel (p,q,n) at _packed_pos;
    # bias on indicator row, -BIG on inverse-indicator row.  Sentinel
    # denominator channel at tile 5 row 72: exp = ~0 inside the image,
    # ~1e30 outside (kills invalid pixels through the reciprocal).
    wa = np.zeros((66, 6 * 128), np.float32)
    for p in range(9):
        for q in range(9):
            for n in range(HEADS):
                row = n * 81 + p * 9 + q
                t, r = _packed_pos(p, q, n)
                wa[:64, t * 128 + r] = w_attn[row] * sc
                wa[64, t * 128 + r] = b_attn[row] * sc
                wa[65, t * 128 + r] = -BIG
    wa[64, 5 * 128 + 72] = -BIG
    wa[65, 5 * 128 + 72] = LNBIGD
    add("wa", wa)
    # bct: per tile, [72 -> 128] replicating compact rden (p,n) into the
    # packed layout (zero for pad channels).
    bct = np.zeros((72, 6 * 128), np.float32)
    for p in range(9):
        for q in range(9):
            for n in range(HEADS):
                t, r = _packed_pos(p, q, n)
                bct[8 * p + n, t * 128 + r] = 1.0
    add("bct", bct)
    idm = np.zeros((72, 64), np.float32)
    idm[:64] = np.eye(64, dtype=np.float32)
    add("id64", idm)
    add("wp", w_proj.T)
    bpi = np.zeros((66, 64), np.float32)
    bpi[64] = b_proj                             # indicator row
    bpi[65] = -LARGE
    add("bpi", bpi)
    bpineg = np.zeros((66, 64), np.float32)
    bpineg[64] = -b_proj
    bpineg[65] = -LARGE
    add("bpineg", bpineg)

    # ---- 128-partition weights (wts2) ----
    cols2, parts2 = {}, []

    def add2(name, a):
        a = np.asarray(a, np.float32)
        full = np.zeros((128, a.shape[1]), np.float32)
        full[:a.shape[0]] = a
        cols2[name] = sum(p.shape[1] for p in parts2)
        parts2.append(full)

    # fold128: [128, 64], sums upper+lower 64-halves
    fold = np.zeros((128, 64), np.float32)
    fold[:64] = np.eye(64, dtype=np.float32)
    fold[64:] = np.eye(64, dtype=np.float32)
    add2("fold", fold)
    cpair = np.zeros((128, 4 * 3 * 64), np.float32)
    for k in range(4):
        for j, dj in enumerate((-1, 0, 1)):
            cpair[0:64, (k * 3 + j) * 64:(k * 3 + j) * 64 + 64] = \
                wtap(k, 0, dj)
            cpair[64:128, (k * 3 + j) * 64:(k * 3 + j) * 64 + 64] = \
                wtap(k, -1, dj)
    add2("cpair", cpair)
    # den6: per tile, [128 -> 72] compact softmax denominator (p,n);
    # sentinel channel (t5 row 72) feeds every (p,n).
    den6 = np.zeros((128, 6 * 72), np.float32)
    for p in range(9):
        for q in range(9):
            for n in range(HEADS):
                t, r = _packed_pos(p, q, n)
                den6[r, t * 72 + 8 * p + n] = 1.0
    den6[72, 5 * 72:6 * 72] = 1.0
    add2("den6", den6)
    # G-build: per (p, seg): [128, 128] -> G0 and/or [128, 72] -> G1
    gb0 = np.zeros((128, 0), np.float32)
    g0blocks, g1blocks = [], []
    for (p, t, has_g0, has_g1) in GB_SEGS:
        w0 = np.zeros((128, 128), np.float32)
        w1 = np.zeros((128, 72), np.float32)
        for (tt, r0, lo, hi) in P_SEGS[p]:
            if tt != t:
                continue
            for ch in range(lo, hi):
                q, n = divmod(ch, 8)
                oc = 8 * _s_idx(p, q) + n
                if oc < 128:
                    w0[r0 + (ch - lo), oc] = 1.0
                else:
                    w1[r0 + (ch - lo), oc - 128] = 1.0
        if has_g0:
            g0blocks.append(w0)
        if has_g1:
            g1blocks.append(w1)
    add2("gb0", np.concatenate(g0blocks, axis=1))
    add2("gb1", np.concatenate(g1blocks, axis=1))
    # sel: per group a [128, 128] block; weight rows sit inside the aligned
    # contract window [base, base+csz) matching the G-tile rows read.
    selw = np.zeros((128, 13 * 128), np.float32)
    for g_i, g in enumerate(SEL_GROUPS):
        src, bp, csz = SEL_WIN[g_i]
        r0 = 8 * g[0] - 128 * src
        for j, si in enumerate(g):
            for n in range(HEADS):
                for dd in range(HD):
                    selw[r0 + 8 * (si - g[0]) + n,
                         g_i * 128 + 64 * j + 8 * n + dd] = 1.0
    add2("sel", selw)
    return (np.concatenate(parts, axis=1), cols,
            np.concatenate(parts2, axis=1), cols2)


def _build(wcols, wtotal, wcols2, wtotal2):
    nc = bass.Bass("TRN2", target_bir_lowering=False, debug=False)
    xs = nc.dram_tensor("xs", [66, NPX], F32, kind="ExternalInput").ap()
    wts = nc.dram_tensor("wts", [72, wtotal], F32, kind="ExternalInput").ap()
    wts2 = nc.dram_tensor("wts2", [128, wtotal2], F32,
                          kind="ExternalInput").ap()
    y = nc.dram_tensor("y", [C, 64 * W], F32, kind="ExternalOutput").ap()

    def wr_at(w, name, r0, r1, n=64):
        return w[r0:r1, wcols[name]:wcols[name] + n]

    with _TCtx(nc) as tc:
        cpool = tc.alloc_tile_pool(name="const", bufs=1)
        spool = tc.alloc_tile_pool(name="stage", bufs=2)
        rpool_ = tc.alloc_tile_pool(name="raw", bufs=1, side="right")

        wf = rpool_.tile([72, wtotal], F32, tag="wf")
        nc.sync.dma_start(wf[:], wts[:])
        wr = cpool.tile([72, wtotal], BF16)
        nc.vector.tensor_copy(wr[:], wf[:])
        w2f = rpool_.tile([128, wtotal2], F32, tag="w2f")
        nc.sync.dma_start(w2f[:], wts2[:])
        w2 = cpool.tile([128, wtotal2], BF16)
        nc.vector.tensor_copy(w2[:], w2f[:])
        fold128 = w2[:, wcols2["fold"]:wcols2["fold"] + 64]
        wb = cpool.tile([72, 192], BF16)
        nc.vector.tensor_copy(wb[:, 0:64], wf[:, wcols["wp"]:wcols["wp"] + 64])
        nc.vector.tensor_copy(wb[:, 64:128],
                              wf[:, wcols["id64"]:wcols["id64"] + 64])
        nc.vector.tensor_scalar_mul(wb[:, 128:192],
                                    wf[:, wcols["wp"]:wcols["wp"] + 64], -1.0)

        ind2 = cpool.tile([2, NPX], BF16)

        def conv_matmuls(ps, src, k, o, n):
            first = True
            for j in range(3):
                cp = wcols2["cpair"] + (k * 3 + j) * 64
                nc.tensor.matmul(ps[:, :n], w2[0:128, cp:cp + 64],
                                 src[0:128, o + j - 1:o + j - 1 + n],
                                 start=first, stop=False)
                first = False
            for j in range(3):
                cs = wcols["csing"] + (k * 3 + j) * 64
                nc.tensor.matmul(ps[:, :n], wr[0:64, cs:cs + 64],
                                 src[0:64, o + PW + j - 1:o + PW + j - 1 + n],
                                 start=False, stop=False)
            cb = wcols["cbias"] + k * 64
            nc.tensor.matmul(ps[:, :n], wr[0:2, cb:cb + 64],
                             ind2[0:2, o:o + n], start=False, stop=True)

        def conv_stage(cps, src, dst, k, r0, r1, stack):
            lo, hi = r0 * PW + 1, r1 * PW - 1
            for o in range(lo, hi, CVN):
                n = min(CVN, hi - o)
                ps = cps.tile([64, CVN], F32)
                conv_matmuls(ps, src, k, o, n)
                nc.scalar.activation(dst[0:64, o:o + n], ps[:, :n], AF.Relu)
                if stack:
                    nc.sync.dma_start(dst[64:128, o + PW:o + PW + n],
                                      dst[0:64, o:o + n])
            # interior padding cols come out exactly 0 from the -LARGE
            # indicator clamp + ReLU; only the two corner elements outside
            # the evacuation span [lo, hi) need explicit zeroing.
            du = dst[0:64].bitcast(U16)
            nc.gpsimd.memset(du[:, r0 * PW:r0 * PW + 1], 0.0)
            nc.gpsimd.memset(du[:, r1 * PW - 1:r1 * PW], 0.0)
            if stack:
                dl = dst[64:128].bitcast(U16)
                nc.gpsimd.memset(dl[:, (r0 + 1) * PW:(r0 + 1) * PW + 1], 0.0)
                nc.gpsimd.memset(dl[:, (r1 + 1) * PW - 1:(r1 + 1) * PW], 0.0)

        # ---- input & convs 1,2 ----
        x0f = rpool_.tile([66, NPX], F32, tag="x0f")
        nc.sync.dma_start(x0f[:], xs[:])
        x0s = spool.tile([128, NPX], BF16, tag="sbig")
        nc.vector.tensor_copy(ind2[:], x0f[64:66, :])
        NQ = NPX // 4
        for j in range(4):
            a, b = j * NQ, NPX if j == 3 else (j + 1) * NQ
            nc.vector.tensor_copy(x0s[0:64, a:b], x0f[0:64, a:b])
            nc.sync.dma_start(x0s[64:128, a + PW:b + PW] if j < 3 else
                              x0s[64:128, a + PW:NPX],
                              x0s[0:64, a:b] if j < 3 else
                              x0s[0:64, a:NPX - PW])
        rpool_.release()
        x1s = spool.tile([128, NPX], BF16, tag="sbig")
        x2 = spool.tile([66, NPX], BF16, tag="stage", bufs=1)
        nc.sync.dma_start(x2[64:66, :], ind2[0:2, :])
        with tc.tile_pool(name="cps", bufs=4, space="PSUM") as cps:
            conv_stage(cps, x0s, x1s, 0, 2, 76, stack=True)
            conv_stage(cps, x1s, x2, 1, 3, 75, stack=False)

        # ---- attention (s-space) ----
        z = spool.tile([128, NPX], BF16, tag="sbig")
        vfull = spool.tile([128, NPX], BF16, tag="vf", bufs=1)
        nc.gpsimd.memset(vfull[:].bitcast(U16), 0.0)

        GR0, GR1 = 4, 74
        ZR0, ZR1 = 5, 73
        n_chunks = (GR1 - GR0 + 2) // 3

        apool = tc.alloc_tile_pool(name="ap", bufs=2)
        epool = tc.alloc_tile_pool(name="ep", bufs=2)
        rpool = tc.alloc_tile_pool(name="rd", bufs=2)
        tpool = tc.alloc_tile_pool(name="tmp", bufs=2)
        gpool_s = tc.alloc_tile_pool(name="gb", bufs=2)
        ypool = tc.alloc_tile_pool(name="yb", bufs=2)
        mps = tc.alloc_tile_pool(name="mps", bufs=2, space="PSUM")
        vy = tc.alloc_tile_pool(name="vy", bufs=1, space="PSUM")
        sps = tc.alloc_tile_pool(name="sps", bufs=2, space="PSUM")
        gps = tc.alloc_tile_pool(name="gps", bufs=1, space="PSUM")

        a_tiles = {}

        def chunk_rows(ci):
            t0 = GR0 + 3 * ci
            return t0, min(t0 + 3, GR1)

        def do_a(ci):
            t0, t1 = chunk_rows(ci)
            S0 = t0 * PW + 1
            SN = (t1 - t0) * PW - 2
            # v rows [3+3ci, min(6+3ci, 75)) into persistent mirrored vfull
            rv0, rv1 = 3 + 3 * ci, min(6 + 3 * ci, 75)
            vlo, vn = rv0 * PW + 1, (rv1 - rv0) * PW - 2
            psv = mps.tile([64, CONVN], F32, tag="m")
            nc.tensor.matmul(psv[:, :vn], wr_at(wr, "wv", 0, 65),
                             x2[0:65, vlo:vlo + vn], start=True, stop=True)
            nc.scalar.copy(vfull[0:64, vlo:vlo + vn], psv[:, :vn])
            nc.sync.dma_start(vfull[64:128, vlo:vlo + vn],
                              vfull[0:64, vlo:vlo + vn])
            # packed logits -> exp
            E = []
            for i in range(6):
                psl = mps.tile([128, CONVN], F32, tag="m")
                nc.tensor.matmul(psl[:, :SN],
                                 wr[0:66, wcols["wa"] + i * 128:
                                    wcols["wa"] + i * 128 + 128],
                                 x2[0:66, S0:S0 + SN], start=True, stop=True)
                e = epool.tile([128, CONVN], BF16, tag=f"E{i}")
                nc.scalar.activation(e[:, :SN], psl[:, :SN], AF.Exp)
                E.append(e)
            # compact denominator (p,n) + fast reciprocal
            psd = mps.tile([128, CONVN], F32, tag="m")
            for i in range(6):
                nc.tensor.matmul(psd[0:72, :SN],
                                 w2[0:128, wcols2["den6"] + i * 72:
                                    wcols2["den6"] + i * 72 + 72],
                                 E[i][:, :SN], start=(i == 0), stop=(i == 5))
            rdc = rpool.tile([72, CONVN], F32, tag="rdc")
            nc.vector.reciprocal(rdc[:, :SN], psd[0:72, :SN])
            rdcb = rpool.tile([72, CONVN], BF16, tag="rdcb")
            nc.gpsimd.tensor_copy(rdcb[:, :SN], rdc[:, :SN])
            # A tiles (normalized attention field) with 1-row halo layout:
            # [128, 5PW], own rows at [PW, (1+rows)*PW)
            for i in range(6):
                psr = mps.tile([128, CONVN], F32, tag="m")
                nc.tensor.matmul(psr[:, :SN],
                                 wr[0:72, wcols["bct"] + i * 128:
                                    wcols["bct"] + i * 128 + 128],
                                 rdcb[:, :SN], start=True, stop=True)
                at = apool.tile([128, 5 * PW], BF16, tag=f"A{i}")
                nc.vector.tensor_tensor(at[:, PW + 1:PW + 1 + SN],
                                        psr[:, :SN], E[i][:, :SN], OP.mult)
                atu = at[:].bitcast(U16)
                if ci == 0:
                    nc.gpsimd.memset(atu[:, 0:PW + 1], 0.0)
                else:
                    nc.gpsimd.memset(atu[:, PW:PW + 1], 0.0)
                nc.gpsimd.memset(atu[:, PW + 1 + SN:PW + 2 + SN], 0.0)
                a_tiles[(ci, i)] = at
            # halo exchange with previous chunk (first/last own rows)
            if ci > 0:
                for i in range(6):
                    prev = a_tiles[(ci - 1, i)]
                    cur = a_tiles[(ci, i)]
                    nc.gpsimd.tensor_copy(cur[:, 0:PW], prev[:, 3 * PW:4 * PW])
                    nc.gpsimd.tensor_copy(prev[:, 4 * PW:5 * PW],
                                          cur[:, PW:2 * PW])
                    a_tiles.pop((ci - 2, i), None)

        def do_z(ci):
            t0, t1 = chunk_rows(ci)
            zr0, zr1 = max(t0, ZR0), min(t1, ZR1)
            if zr0 >= zr1:
                return
            zS0 = zr0 * PW + 1
            SN = (zr1 - zr0) * PW - 2
            base = (t0 - 1) * PW
            # G-build: 18 shifted 0/1 matmuls accumulating (s,n) channels
            g0p = gps.tile([128, CONVN], F32, tag="g0")
            g1p = gps.tile([72, CONVN], F32, tag="g1")
            n0 = n1 = 0
            i0 = sum(1 for (_, _, h0, _) in GB_SEGS if h0)
            i1 = sum(1 for (_, _, _, h1) in GB_SEGS if h1)
            for (p, t, has_g0, has_g1) in GB_SEGS:
                di, dj = TAPS[p]
                off = di * PW + dj
                la = zS0 - off - base
                at = a_tiles[(ci, t)]
                if has_g0:
                    c0 = wcols2["gb0"] + n0 * 128
                    nc.tensor.matmul(g0p[:, :SN], w2[0:128, c0:c0 + 128],
                                     at[:, la:la + SN],
                                     start=(n0 == 0), stop=(n0 == i0 - 1))
                    n0 += 1
                if has_g1:
                    c1 = wcols2["gb1"] + n1 * 72
                    nc.tensor.matmul(g1p[:, :SN], w2[0:128, c1:c1 + 72],
                                     at[:, la:la + SN],
                                     start=(n1 == 0), stop=(n1 == i1 - 1))
                    n1 += 1
            g0b = gpool_s.tile([128, CONVN], BF16, tag="g0b")
            nc.scalar.copy(g0b[:, :SN], g0p[:, :SN])
            g1b = gpool_s.tile([72, CONVN], BF16, tag="g1b")
            nc.scalar.copy(g1b[:, :SN], g1p[:, :SN])
            # sel expand -> multiply v -> fold into y
            psy = vy.tile([64, CONVN], F32, tag="y")
            for g_i, g in enumerate(SEL_GROUPS):
                wid = 64 * len(g)
                src, bp, csz = SEL_WIN[g_i]
                scol = wcols2["sel"] + g_i * 128
                gt = g0b if src == 0 else g1b
                psp = sps.tile([128, CONVN], F32, tag="s")
                nc.tensor.matmul(psp[0:wid, :SN],
                                 w2[bp:bp + csz, scol:scol + wid],
                                 gt[bp:bp + csz, :SN],
                                 start=True, stop=True)
                tm = tpool.tile([128, CONVN], BF16, tag=f"T{g_i}")
                o_up = zS0 + S_OFF[g[0]][0] * PW + S_OFF[g[0]][1]
                nc.vector.tensor_tensor(tm[0:64, :SN], psp[0:64, :SN],
                                        vfull[0:64, o_up:o_up + SN], OP.mult)
                if len(g) == 2:
                    o_lo = zS0 + S_OFF[g[1]][0] * PW + S_OFF[g[1]][1]
                    nc.vector.tensor_tensor(tm[64:128, :SN], psp[64:128, :SN],
                                            vfull[64:128, o_lo:o_lo + SN],
                                            OP.mult)
                    nc.tensor.matmul(psy[:, :SN], fold128[:, :], tm[:, :SN],
                                     start=(g_i == 0), stop=False)
                else:
                    nc.tensor.matmul(psy[:, :SN], wb[0:64, 64:128],
                                     tm[0:64, :SN], start=False, stop=True)
            yb = ypool.tile([64, CONVN], BF16, tag="yb")
            nc.scalar.copy(yb[:, :SN], psy[:, :SN])

            def proj(psz, wcol, bias_name):
                nc.tensor.matmul(psz[:, :SN], wb[0:64, wcol:wcol + 64],
                                 yb[:, :SN], start=True, stop=False)
                nc.tensor.matmul(psz[:, :SN], wr_at(wr, bias_name, 64, 66),
                                 x2[64:66, zS0:zS0 + SN],
                                 start=False, stop=True)

            psz = vy.tile([64, CONVN], F32, tag="z")
            proj(psz, 0, "bpi")
            # rows [ZR0, 7) and [71, ZR1) may be out-of-image (which side
            # depends on the half); clamp those z pixels to 0 via
            # relu(psz - L*invind) - relu(-psz - L*invind).
            edge = [(r0, r1) for (r0, r1) in ((zr0, min(zr1, 7)),
                                              (max(zr0, 71), zr1))
                    if r0 < r1]
            if edge:
                pszn = mps.tile([64, CONVN], F32, tag="m")
                proj(pszn, 128, "bpineg")
                for (r0, r1) in edge:
                    e0 = r0 * PW + 1 - zS0
                    en = (r1 - r0) * PW - 2
                    tp = tpool.tile([64, CONVN], F32, tag="zp", bufs=1)
                    nc.scalar.activation(tp[:, :en], psz[:, e0:e0 + en],
                                         AF.Relu)
                    tn = tpool.tile([64, CONVN], F32, tag="zn", bufs=1)
                    nc.scalar.activation(tn[:, :en], pszn[:, e0:e0 + en],
                                         AF.Relu)
                    nc.vector.tensor_tensor(
                        z[0:64, r0 * PW + 1:r0 * PW + 1 + en],
                        tp[:, :en], tn[:, :en], OP.subtract)
            m0, m1 = max(zr0, 7), min(zr1, 71)
            if m0 < m1:
                e0 = m0 * PW + 1 - zS0
                en = (m1 - m0) * PW - 2
                nc.scalar.copy(z[0:64, m0 * PW + 1:m0 * PW + 1 + en],
                               psz[:, e0:e0 + en])
            # mirror the written rows into the stacked lower half
            nc.sync.dma_start(z[64:128, zS0 + PW:zS0 + PW + SN],
                              z[0:64, zS0:zS0 + SN])

        for ci in range(n_chunks):
            do_a(ci)
            if ci >= 1:
                do_z(ci - 1)
        do_z(n_chunks - 1)
        z3 = z[0:64].bitcast(U16).rearrange("c (r w) -> c r w", r=TROWS, w=PW)
        nc.gpsimd.memset(z3[:, ZR0:ZR1, 0:1], 0.0)
        nc.gpsimd.memset(z3[:, ZR0:ZR1, PW - 1:PW], 0.0)
        z3l = z[64:128].bitcast(U16).rearrange("c (r w) -> c r w",
                                               r=TROWS, w=PW)
        nc.gpsimd.memset(z3l[:, ZR0 + 1:ZR1 + 1, 0:1], 0.0)
        nc.gpsimd.memset(z3l[:, ZR0 + 1:ZR1 + 1, PW - 1:PW], 0.0)

        for pool in (gps, sps, vy, mps, ypool, gpool_s, tpool, rpool,
                     epool, apool):
            pool.release()

        # ---- convs 3,4 and output ----
        y3s = spool.tile([128, NPX], BF16, tag="sbig")
        yepool = tc.alloc_tile_pool(name="ye", bufs=2)
        with tc.tile_pool(name="cps2", bufs=4, space="PSUM") as cps2:
            conv_stage(cps2, z, y3s, 2, 6, 72, stack=True)
            # conv4: 3-image-row windows evacuated straight to the output
            y3d = y[:].rearrange("c (r w) -> c r w", r=64, w=W)
            for r in range(7, 71, 3):
                nr = min(3, 71 - r)
                o = r * PW + 1
                n = nr * PW - 2
                ps = cps2.tile([64, CVN], F32)
                conv_matmuls(ps, y3s, 3, o, n)
                yb = yepool.tile([64, 390], F32, tag="yb")
                nc.scalar.activation(yb[:, :n], ps[:, :n], AF.Relu)
                ybv = AP(yb.tensor, yb.offset,
                         [[390, 64], [PW, nr], [1, W]])
                nc.sync.dma_start(y3d[:, r - 7:r - 7 + nr, :], ybv)
        yepool.release()
        spool.release()
        cpool.release()
    return nc


_CACHE = {}
TRACE_DIR = None
LAST_EXEC_NS = None


def kernel(**inputs):
    x = np.asarray(inputs["x"], np.float32)
    conv_w = np.asarray(inputs["conv_w"], np.float32)
    args = (conv_w, np.asarray(inputs["bn_g"], np.float32),
            np.asarray(inputs["bn_b"], np.float32),
            np.asarray(inputs["bn_m"], np.float32),
            np.asarray(inputs["bn_v"], np.float32),
            np.asarray(inputs["w_v"], np.float32),
            np.asarray(inputs["b_v"], np.float32),
            np.asarray(inputs["w_attn"], np.float32),
            np.asarray(inputs["b_attn"], np.float32),
            np.asarray(inputs["w_proj"], np.float32),
            np.asarray(inputs["b_proj"], np.float32))
    wts, wcols, wts2, wcols2 = _prep_weights(*args)

    # per-core input shards with indicator channels
    shards = []
    for core in range(N_CORES):
        bb, half = divmod(core, 2)
        s = half * 64
        rowbase = s - 7
        sh = np.zeros((66, TROWS, PW), np.float32)
        r0, r1 = max(0, rowbase + 1), min(H, rowbase + 77)
        sh[:64, r0 - rowbase:r1 - rowbase, 1:1 + W] = x[bb, :, r0:r1, :]
        sh[64, r0 - rowbase:r1 - rowbase, 1:1 + W] = 1.0
        sh[65] = 1.0 - sh[64]
        shards.append(sh.reshape(66, NPX))

    key = "k1"
    if key not in _CACHE:
        _CACHE[key] = _build(wcols, wts.shape[1], wcols2, wts2.shape[1])
    nc = _CACHE[key]
    in_maps = [{"xs": shards[i], "wts": wts, "wts2": wts2}
               for i in range(N_CORES)]
    kw = {}
    if TRACE_DIR is not None:
        kw = dict(trace=True, tmpdir=TRACE_DIR)
    res = run_bass_kernel_spmd(nc, in_maps, core_ids=list(range(N_CORES)), **kw)
    global LAST_EXEC_NS
    LAST_EXEC_NS = res.exec_time_ns
    return out_from(res)


def out_from(res):
    out = np.zeros((B, C, H, W), np.float32)
    for core in range(N_CORES):
        bb, half = divmod(core, 2)
        s = half * 64
        out[bb, :, s:s + 64, :] = res.results[core]["y"].reshape(C, 64, W)
    return out


# revision 26
# speedup vs baseline: 1.4093x; 1.1255x over previous
"""ContrastAwareAttentionBlock Trainium2 Bass kernel (s-space restructure).

Sharding: 8 cores = (batch 4) x (image half: rows 0-63 / 64-127); each core
computes its half with a 6-row halo of redundant compute — no collectives.

Attention is computed in "s-space": with s = off_q - off_p (25 values in a
5x5 grid), fold+proj collapse to  z[:,l] = w_proj @ sum_s G_s[n,l] *
v[:, l+s] + b_proj  where  G_s[n,l] = sum_p A[n, l-off_p, p, off_p+s]  and
A is the normalized attention field (packed (p,q,n) channels, 6 tiles of
128).  Per 3-row chunk this needs only 18 G-build matmuls (0/1 selectors
over row-shifted A tiles, accumulated in 2 PSUM tiles of 200 (s,n)
channels), 13 sel expansions ((s,n) -> (s,n,d)), 26 half-DVE multiplies
against a persistent mirrored v field, 13 fold matmuls and a single local
projection — vs 45 sel + 45 fold + 10 shifted-proj matmuls in the direct
(p,q) form.  Invalid pixels die via indicator channels: logits get -BIG,
the softmax denominator gets a huge sentinel channel (exp(ln BIGD) on the
inverse indicator), so A ~ 0 outside the image and all shifted reads are
harmless.  Convs are unchanged: 3x3 conv = 9 PSUM-accumulated matmuls over
shifted slices with BN folded into weights and an indicator-channel bias.
"""
import sys
sys.path.insert(0, '/opt/trn_rl_repo')
import numpy as np

import concourse.bass as bass
import concourse.tile as tile
from concourse import mybir
from concourse.ap import AP
from concourse.bass_utils import run_bass_kernel_spmd

F32 = mybir.dt.float32
BF16 = mybir.dt.bfloat16
U16 = mybir.dt.uint16
AF = mybir.ActivationFunctionType
OP = mybir.AluOpType

B, C, H, W, HEADS, HD = 4, 64, 128, 128, 8, 8
PW = W + 2
TROWS = 78            # tile rows map to image rows [s-7, s+71)
NPX = TROWS * PW
N_CORES = 8
BN_EPS = 1e-5
BIG = 80.0            # exp(-BIG) ~ 0 for invalid pixels
BIGD = 1e30           # denominator for invalid pixels
LARGE = 1e30          # relu clamp for invalid pixels
CONVN = 390           # attention matmul chunk (3 rows, 1 PSUM bank)
CVN = 512             # conv matmul chunk (full PSUM bank)
MAX_WAITS = 1
sc = HD ** -0.5

TAPS = [(di, dj) for di in (-1, 0, 1) for dj in (-1, 0, 1)]

# packed-E layout: 6 tiles x 128; p0..p5 whole, p6,p7,p8 split
P_SEGS = {
    0: [(0, 0, 0, 72)],
    1: [(1, 16, 0, 72)],
    2: [(2, 32, 0, 72)],
    3: [(3, 48, 0, 72)],
    4: [(4, 0, 0, 72)],
    5: [(5, 0, 0, 72)],
    6: [(0, 72, 0, 56), (1, 0, 56, 72)],
    7: [(1, 88, 0, 40), (2, 0, 40, 72)],
    8: [(2, 104, 0, 24), (3, 0, 24, 72)],
}


def _packed_pos(p, q, n):
    ch = 8 * q + n
    for (t, r0, lo, hi) in P_SEGS[p]:
        if lo <= ch < hi:
            return t, r0 + (ch - lo)
    raise AssertionError


def _s_idx(p, q):
    si = TAPS[q][0] - TAPS[p][0]
    sj = TAPS[q][1] - TAPS[p][1]
    return (si + 2) * 5 + (sj + 2)


S_OFF = [((i // 5) - 2, (i % 5) - 2) for i in range(25)]
# pairs are horizontally adjacent s values (one full-width DVE multiply
# against [v; v<<1col]); the per-s-row leftovers are singles stacked into
# shared T tiles (ST0: 4+9, ST1: 14+15, ST2: 24).
SEL_GROUPS = [(0, 1), (2, 3), (4,), (5, 6), (7,), (8, 9), (10, 11),
              (12, 13), (14,), (15,), (16, 17), (18, 19), (20, 21),
              (22, 23), (24,)]
# sel matmul windows: (src 0=G0b/1=G1b, base partition, size); matmul
# partition windows must start at 0/32/64 and stay inside the quadrant
# (base 32: <=32 rows, base 64: <=64 rows); lhsT base must match rhs.
SEL_WIN = [(0, 0, 32), (0, 0, 32), (0, 32, 8), (0, 32, 32), (0, 32, 32),
           (0, 64, 32), (0, 64, 32), (0, 64, 64), (0, 64, 64), (0, 64, 64),
           (1, 0, 32), (1, 0, 32), (1, 32, 32), (1, 32, 32), (1, 64, 8)]
# single groups: (group idx, T-tile id, half 0=up/1=low)
SINGLE_T = {2: (10, 0), 4: (10, 1), 8: (11, 0), 9: (11, 1), 14: (12, 0)}
# G-build segments: (p, tile, has_g0, has_g1); built to match P_SEGS
GB_SEGS = []
for _p in range(9):
    for (_t, _r0, _lo, _hi) in P_SEGS[_p]:
        _g0 = any(8 * _s_idx(_p, ch // 8) + ch % 8 < 128
                  for ch in range(_lo, _hi))
        _g1 = any(8 * _s_idx(_p, ch // 8) + ch % 8 >= 128
                  for ch in range(_lo, _hi))
        GB_SEGS.append((_p, _t, _g0, _g1))


def _split_excess_waits(nc):
    """This walrus build rejects >1 sync wait per instruction; move excess
    waits onto same-engine NOPs inserted before the offender."""
    bbs, fixups = [], {}
    for f in nc.m.functions:
        for bb in f.blocks:
            bbs.append(bb)
            for inst in bb.instructions:
                si = inst.sync_info
                waits = list(si.on_wait) if si is not None and si.on_wait else []
                if len(waits) > MAX_WAITS:
                    si.on_wait = waits[:MAX_WAITS]
                    rest = waits[MAX_WAITS:]
                    chunks = [rest[i:i + MAX_WAITS]
                              for i in range(0, len(rest), MAX_WAITS)]
                    fixups.setdefault(id(bb), {}).setdefault(inst.name, []).extend(
                        (inst.engine, ch) for ch in chunks)
    if not fixups:
        return
    created = {}
    for bb in bbs:
        for name, specs in fixups.get(id(bb), {}).items():
            nops = []
            for engine, ch in specs:
                bi = nc.engines[engine].nop(nofuse=True)
                bi.ins.sync_info = mybir.SyncInfo(on_wait=ch, on_update=[])
                nops.append(bi.ins)
            created[name] = nops
    all_nops = {n.name for ns in created.values() for n in ns}
    for bb in bbs:
        insts = [i for i in bb.instructions if i.name not in all_nops]
        new = []
        for inst in insts:
            new.extend(created.get(inst.name, ()))
            new.append(inst)
        bb.instructions = new


class _TCtx(tile.TileContext):
    def _drain_and_barrier(self, tick_clock, wait_clock):
        from concourse.tile import ScopedClock
        probe = self.nc.sync.nop(nofuse=True)
        wait_clock.add_sem_waits(
            probe.ins, ScopedClock({None: tick_clock.global_clock}))
        self.nc.sync.drain()
        self.nc.all_engine_barrier()
        assert self.sems is not None
        popped = self.nc._tile_sem_poison_stack.pop()
        assert popped is self._sem_poison
        self.nc.clear_and_free_semaphores(list(self.sems.allocated().values()))
        self.nc.all_engine_barrier()

    def __exit__(self, exc_type, exc_val, exc_tb):
        ret = super().__exit__(exc_type, exc_val, exc_tb)
        if exc_type is None:
            _split_excess_waits(self.nc)
        return ret


def _prep_weights(conv_w, bn_g, bn_b, bn_m, bn_v, w_v, b_v, w_attn, b_attn,
                  w_proj, b_proj):
    inv = bn_g / np.sqrt(bn_v + BN_EPS)          # [4, 64]
    beta = bn_b - bn_m * inv                     # [4, 64]
    cols, parts = {}, []

    def add(name, a):
        a = np.asarray(a, np.float32)
        full = np.zeros((72, a.shape[1]), np.float32)
        full[:a.shape[0]] = a
        cols[name] = sum(p.shape[1] for p in parts)
        parts.append(full)

    # conv taps: pairs {(0,dj),(-1,dj)} contract 128 over the row-stacked
    # input ([x; x shifted one row down]); singles (1,dj) contract 64;
    # beta + the -LARGE out-of-image clamp ride a 2-row indicator matmul.
    def wtap(k, di, dj):
        return conv_w[k, :, :, di + 1, dj + 1].T * inv[k][None, :]

    csing = np.zeros((64, 4 * 3 * 64), np.float32)
    cbias = np.zeros((2, 4 * 64), np.float32)
    for k in range(4):
        for j, dj in enumerate((-1, 0, 1)):
            csing[:, (k * 3 + j) * 64:(k * 3 + j) * 64 + 64] = wtap(k, 1, dj)
        cbias[0, k * 64:k * 64 + 64] = beta[k]
        cbias[1, k * 64:k * 64 + 64] = -LARGE
    add("csing", csing)
    add("cbias", cbias)
    wv65 = np.zeros((65, 64), np.float32)
    wv65[:64] = w_v.T
    wv65[64] = b_v
    add("wv", wv65)
    # packed attention logits [66, 6*128]: channel (p,q,n) at _packed_pos;
    # bias on indicator row, -BIG on inverse-indicator row.
    wa = np.zeros((66, 6 * 128), np.float32)
    for p in range(9):
        for q in range(9):
            for n in range(HEADS):
                row = n * 81 + p * 9 + q
                t, r = _packed_pos(p, q, n)
                wa[:64, t * 128 + r] = w_attn[row] * sc
                wa[64, t * 128 + r] = b_attn[row] * sc
                wa[65, t * 128 + r] = -BIG
    add("wa", wa)
    bigd72 = np.zeros((66, 72), np.float32)
    bigd72[65] = BIGD                            # inverse-indicator row
    add("bigd72", bigd72)
    # bct: per tile, [72 -> 128] replicating compact rden (p,n) into the
    # packed layout (zero for pad channels).
    bct = np.zeros((72, 6 * 128), np.float32)
    for p in range(9):
        for q in range(9):
            for n in range(HEADS):
                t, r = _packed_pos(p, q, n)
                bct[8 * p + n, t * 128 + r] = 1.0
    add("bct", bct)
    idm = np.zeros((72, 64), np.float32)
    idm[:64] = np.eye(64, dtype=np.float32)
    add("id64", idm)
    add("wp", w_proj.T)
    bpi = np.zeros((66, 64), np.float32)
    bpi[64] = b_proj                             # indicator row
    bpi[65] = -LARGE
    add("bpi", bpi)
    bpineg = np.zeros((66, 64), np.float32)
    bpineg[64] = -b_proj
    bpineg[65] = -LARGE
    add("bpineg", bpineg)

    # ---- 128-partition weights (wts2) ----
    cols2, parts2 = {}, []

    def add2(name, a):
        a = np.asarray(a, np.float32)
        full = np.zeros((128, a.shape[1]), np.float32)
        full[:a.shape[0]] = a
        cols2[name] = sum(p.shape[1] for p in parts2)
        parts2.append(full)

    # fold128: [128, 64], sums upper+lower 64-halves
    fold = np.zeros((128, 64), np.float32)
    fold[:64] = np.eye(64, dtype=np.float32)
    fold[64:] = np.eye(64, dtype=np.float32)
    add2("fold", fold)
    cpair = np.zeros((128, 4 * 3 * 64), np.float32)
    for k in range(4):
        for j, dj in enumerate((-1, 0, 1)):
            cpair[0:64, (k * 3 + j) * 64:(k * 3 + j) * 64 + 64] = \
                wtap(k, 0, dj)
            cpair[64:128, (k * 3 + j) * 64:(k * 3 + j) * 64 + 64] = \
                wtap(k, -1, dj)
    add2("cpair", cpair)
    # den6: per tile, [128 -> 72] compact softmax denominator (p,n)
    den6 = np.zeros((128, 6 * 72), np.float32)
    for p in range(9):
        for q in range(9):
            for n in range(HEADS):
                t, r = _packed_pos(p, q, n)
                den6[r, t * 72 + 8 * p + n] = 1.0
    add2("den6", den6)
    # G-build: per (p, seg): [128, 128] -> G0 and/or [128, 72] -> G1
    gb0 = np.zeros((128, 0), np.float32)
    g0blocks, g1blocks = [], []
    for (p, t, has_g0, has_g1) in GB_SEGS:
        w0 = np.zeros((128, 128), np.float32)
        w1 = np.zeros((128, 72), np.float32)
        for (tt, r0, lo, hi) in P_SEGS[p]:
            if tt != t:
                continue
            for ch in range(lo, hi):
                q, n = divmod(ch, 8)
                oc = 8 * _s_idx(p, q) + n
                if oc < 128:
                    w0[r0 + (ch - lo), oc] = 1.0
                else:
                    w1[r0 + (ch - lo), oc - 128] = 1.0
        if has_g0:
            g0blocks.append(w0)
        if has_g1:
            g1blocks.append(w1)
    add2("gb0", np.concatenate(g0blocks, axis=1))
    add2("gb1", np.concatenate(g1blocks, axis=1))
    # sel: per group a [128, 128] block; weight rows sit inside the aligned
    # contract window [base, base+csz) matching the G-tile rows read.
    selw = np.zeros((128, 15 * 128), np.float32)
    for g_i, g in enumerate(SEL_GROUPS):
        src, bp, csz = SEL_WIN[g_i]
        r0 = 8 * g[0] - 128 * src
        for j, si in enumerate(g):
            for n in range(HEADS):
                for dd in range(HD):
                    selw[r0 + 8 * (si - g[0]) + n,
                         g_i * 128 + 64 * j + 8 * n + dd] = 1.0
    add2("sel", selw)
    return (np.concatenate(parts, axis=1), cols,
            np.concatenate(parts2, axis=1), cols2)


def _build(wcols, wtotal, wcols2, wtotal2):
    nc = bass.Bass("TRN2", target_bir_lowering=False, debug=False)
    xs = nc.dram_tensor("xs", [66, NPX], F32, kind="ExternalInput").ap()
    wts = nc.dram_tensor("wts", [72, wtotal], F32, kind="ExternalInput").ap()
    wts2 = nc.dram_tensor("wts2", [128, wtotal2], F32,
                          kind="ExternalInput").ap()
    y = nc.dram_tensor("y", [C, 64 * W], F32, kind="ExternalOutput").ap()
    dbg = None
    if DEBUG_DUMP:
        dbg = nc.dram_tensor("dbg", [128, 3 * NPX + 2 * 5 * PW], F32,
                             kind="ExternalOutput").ap()

    def wr_at(w, name, r0, r1, n=64):
        return w[r0:r1, wcols[name]:wcols[name] + n]

    with _TCtx(nc) as tc:
        cpool = tc.alloc_tile_pool(name="const", bufs=1)
        spool = tc.alloc_tile_pool(name="stage", bufs=2)
        rpool_ = tc.alloc_tile_pool(name="raw", bufs=1, side="right")

        wf = rpool_.tile([72, wtotal], F32, tag="wf")
        nc.sync.dma_start(wf[:], wts[:])
        wr = cpool.tile([72, wtotal], BF16)
        nc.vector.tensor_copy(wr[:], wf[:])
        w2f = rpool_.tile([128, wtotal2], F32, tag="w2f")
        nc.sync.dma_start(w2f[:], wts2[:])
        w2 = cpool.tile([128, wtotal2], BF16)
        nc.vector.tensor_copy(w2[:], w2f[:])
        fold128 = w2[:, wcols2["fold"]:wcols2["fold"] + 64]
        wb = cpool.tile([72, 192], BF16)
        nc.vector.tensor_copy(wb[:, 0:64], wf[:, wcols["wp"]:wcols["wp"] + 64])
        nc.vector.tensor_copy(wb[:, 64:128],
                              wf[:, wcols["id64"]:wcols["id64"] + 64])
        nc.vector.tensor_scalar_mul(wb[:, 128:192],
                                    wf[:, wcols["wp"]:wcols["wp"] + 64], -1.0)

        ind2 = cpool.tile([2, NPX], BF16)

        def conv_matmuls(ps, src, k, o, n):
            first = True
            for j in range(3):
                cp = wcols2["cpair"] + (k * 3 + j) * 64
                nc.tensor.matmul(ps[:, :n], w2[0:128, cp:cp + 64],
                                 src[0:128, o + j - 1:o + j - 1 + n],
                                 start=first, stop=False)
                first = False
            for j in range(3):
                cs = wcols["csing"] + (k * 3 + j) * 64
                nc.tensor.matmul(ps[:, :n], wr[0:64, cs:cs + 64],
                                 src[0:64, o + PW + j - 1:o + PW + j - 1 + n],
                                 start=False, stop=False)
            cb = wcols["cbias"] + k * 64
            nc.tensor.matmul(ps[:, :n], wr[0:2, cb:cb + 64],
                             ind2[0:2, o:o + n], start=False, stop=True)

        def conv_stage(cps, src, dst, k, r0, r1, stack):
            lo, hi = r0 * PW + 1, r1 * PW - 1
            for o in range(lo, hi, CVN):
                n = min(CVN, hi - o)
                ps = cps.tile([64, CVN], F32)
                conv_matmuls(ps, src, k, o, n)
                nc.scalar.activation(dst[0:64, o:o + n], ps[:, :n], AF.Relu)
                if stack:
                    nc.sync.dma_start(dst[64:128, o + PW:o + PW + n],
                                      dst[0:64, o:o + n])
            # interior padding cols come out exactly 0 from the -LARGE
            # indicator clamp + ReLU; only the two corner elements outside
            # the evacuation span [lo, hi) need explicit zeroing.
            du = dst[0:64].bitcast(U16)
            nc.gpsimd.memset(du[:, r0 * PW:r0 * PW + 1], 0.0)
            nc.gpsimd.memset(du[:, r1 * PW - 1:r1 * PW], 0.0)
            if stack:
                dl = dst[64:128].bitcast(U16)
                nc.gpsimd.memset(dl[:, (r0 + 1) * PW:(r0 + 1) * PW + 1], 0.0)
                nc.gpsimd.memset(dl[:, (r1 + 1) * PW - 1:(r1 + 1) * PW], 0.0)

        # ---- input & convs 1,2 ----
        x0f = rpool_.tile([66, NPX], F32, tag="x0f")
        nc.sync.dma_start(x0f[:], xs[:])
        x0s = spool.tile([128, NPX], BF16, tag="sbig")
        nc.vector.tensor_copy(ind2[:], x0f[64:66, :])
        NQ = NPX // 4
        for j in range(4):
            a, b = j * NQ, NPX if j == 3 else (j + 1) * NQ
            nc.vector.tensor_copy(x0s[0:64, a:b], x0f[0:64, a:b])
            nc.sync.dma_start(x0s[64:128, a + PW:b + PW] if j < 3 else
                              x0s[64:128, a + PW:NPX],
                              x0s[0:64, a:b] if j < 3 else
                              x0s[0:64, a:NPX - PW])
        rpool_.release()
        x1s = spool.tile([128, NPX], BF16, tag="sbig")
        x2 = spool.tile([66, NPX], BF16, tag="stage", bufs=1)
        nc.sync.dma_start(x2[64:66, :], ind2[0:2, :])
        with tc.tile_pool(name="cps", bufs=4, space="PSUM") as cps:
            conv_stage(cps, x0s, x1s, 0, 2, 76, stack=True)
            conv_stage(cps, x1s, x2, 1, 3, 75, stack=False)

        # ---- attention (s-space) ----
        z = spool.tile([128, NPX], BF16, tag="sbig")
        vfull = spool.tile([128, NPX], BF16, tag="vf", bufs=1)
        nc.gpsimd.memset(vfull[:].bitcast(U16), 0.0)

        GR0, GR1 = 4, 74
        ZR0, ZR1 = 5, 73
        n_chunks = (GR1 - GR0 + 2) // 3

        apool = tc.alloc_tile_pool(name="ap", bufs=2)
        epool = tc.alloc_tile_pool(name="ep", bufs=2)
        rpool = tc.alloc_tile_pool(name="rd", bufs=2)
        tpool = tc.alloc_tile_pool(name="tmp", bufs=2)
        gpool_s = tc.alloc_tile_pool(name="gb", bufs=2)
        ypool = tc.alloc_tile_pool(name="yb", bufs=2)
        mps = tc.alloc_tile_pool(name="mps", bufs=2, space="PSUM")
        vy = tc.alloc_tile_pool(name="vy", bufs=1, space="PSUM")
        sps = tc.alloc_tile_pool(name="sps", bufs=2, space="PSUM")
        gps = tc.alloc_tile_pool(name="gps", bufs=1, space="PSUM")

        a_tiles = {}

        def chunk_rows(ci):
            t0 = GR0 + 3 * ci
            return t0, min(t0 + 3, GR1)

        z_state = {}

        def a_head(ci):
            t0, t1 = chunk_rows(ci)
            S0 = t0 * PW + 1
            SN = (t1 - t0) * PW - 2
            # v rows [3+3ci, min(6+3ci, 75)) into persistent vfull whose
            # lower half is v shifted one column left (lower[l] = v[l+1])
            rv0, rv1 = 3 + 3 * ci, min(6 + 3 * ci, 75)
            vlo, vn = rv0 * PW + 1, (rv1 - rv0) * PW - 2
            psv = mps.tile([64, CONVN], F32, tag="m")
            nc.tensor.matmul(psv[:, :vn], wr_at(wr, "wv", 0, 65),
                             x2[0:65, vlo:vlo + vn], start=True, stop=True)
            nc.scalar.copy(vfull[0:64, vlo:vlo + vn], psv[:, :vn])
            nc.sync.dma_start(vfull[64:128, vlo - 1:vlo - 1 + vn],
                              vfull[0:64, vlo:vlo + vn])
            # packed logits -> exp
            E = []
            for i in range(6):
                psl = mps.tile([128, CONVN], F32, tag="m")
                nc.tensor.matmul(psl[:, :SN],
                                 wr[0:66, wcols["wa"] + i * 128:
                                    wcols["wa"] + i * 128 + 128],
                                 x2[0:66, S0:S0 + SN], start=True, stop=True)
                e = epool.tile([128, CONVN], BF16, tag=f"E{i}")
                nc.scalar.activation(e[:, :SN], psl[:, :SN], AF.Exp)
                E.append(e)
            return E

        def a_tail(ci, E):
            t0, t1 = chunk_rows(ci)
            SN = (t1 - t0) * PW - 2
            # compact denominator (p,n); reciprocal = exp(-ln(den)) on the
            # ACT engine (ln/exp share one activation table with relu/copy)
            psd = mps.tile([128, CONVN], F32, tag="m")
            for i in range(6):
                nc.tensor.matmul(psd[0:72, :SN],
                                 w2[0:128, wcols2["den6"] + i * 72:
                                    wcols2["den6"] + i * 72 + 72],
                                 E[i][:, :SN], start=(i == 0), stop=False)
            nc.tensor.matmul(psd[0:72, :SN], wr_at(wr, "bigd72", 64, 66, 72),
                             x2[64:66, t0 * PW + 1:t0 * PW + 1 + SN],
                             start=False, stop=True)
            rdc = rpool.tile([72, CONVN], F32, tag="rdc")
            nc.vector.reciprocal(rdc[:, :SN], psd[0:72, :SN])
            rdcb = rpool.tile([72, CONVN], BF16, tag="rdcb")
            nc.gpsimd.tensor_copy(rdcb[:, :SN], rdc[:, :SN])
            # A tiles (normalized attention field) with 1-row halo layout:
            # [128, 5PW], own rows at [PW, (1+rows)*PW)
            for i in range(6):
                psr = mps.tile([128, CONVN], F32, tag="m")
                nc.tensor.matmul(psr[:, :SN],
                                 wr[0:72, wcols["bct"] + i * 128:
                                    wcols["bct"] + i * 128 + 128],
                                 rdcb[:, :SN], start=True, stop=True)
                at = apool.tile([128, 5 * PW], BF16, tag=f"A{i}")
                nc.vector.tensor_tensor(at[:, PW + 1:PW + 1 + SN],
                                        psr[:, :SN], E[i][:, :SN], OP.mult)
                atu = at[:].bitcast(U16)
                if ci == 0:
                    nc.gpsimd.memset(atu[:, 0:PW + 1], 0.0)
                else:
                    nc.gpsimd.memset(atu[:, PW:PW + 1], 0.0)
                nc.gpsimd.memset(atu[:, PW + 1 + SN:PW + 2 + SN], 0.0)
                a_tiles[(ci, i)] = at
            # halo exchange with previous chunk (first/last own rows)
            if ci > 0:
                for i in range(6):
                    prev = a_tiles[(ci - 1, i)]
                    cur = a_tiles[(ci, i)]
                    nc.sync.dma_start(cur[:, 0:PW], prev[:, 3 * PW:4 * PW])
                    nc.sync.dma_start(prev[:, 4 * PW:5 * PW],
                                      cur[:, PW:2 * PW])
                    a_tiles.pop((ci - 2, i), None)

        def z_part1(ci):
            t0, t1 = chunk_rows(ci)
            zr0, zr1 = max(t0, ZR0), min(t1, ZR1)
            if zr0 >= zr1:
                return
            zS0 = zr0 * PW + 1
            SN = (zr1 - zr0) * PW - 2
            base = (t0 - 1) * PW
            # G-build: 18 shifted 0/1 matmuls accumulating (s,n) channels
            g0p = gps.tile([128, CONVN], F32, tag="g0")
            g1p = gps.tile([72, CONVN], F32, tag="g1")
            n0 = n1 = 0
            i0 = sum(1 for (_, _, h0, _) in GB_SEGS if h0)
            i1 = sum(1 for (_, _, _, h1) in GB_SEGS if h1)
            for (p, t, has_g0, has_g1) in GB_SEGS:
                di, dj = TAPS[p]
                off = di * PW + dj
                la = zS0 - off - base
                at = a_tiles[(ci, t)]
                if has_g0:
                    c0 = wcols2["gb0"] + n0 * 128
                    nc.tensor.matmul(g0p[:, :SN], w2[0:128, c0:c0 + 128],
                                     at[:, la:la + SN],
                                     start=(n0 == 0), stop=(n0 == i0 - 1))
                    n0 += 1
                if has_g1:
                    c1 = wcols2["gb1"] + n1 * 72
                    nc.tensor.matmul(g1p[:, :SN], w2[0:128, c1:c1 + 72],
                                     at[:, la:la + SN],
                                     start=(n1 == 0), stop=(n1 == i1 - 1))
                    n1 += 1
            g0b = gpool_s.tile([128, CONVN], BF16, tag="g0b")
            nc.scalar.copy(g0b[:, :SN], g0p[:, :SN])
            g1b = gpool_s.tile([72, CONVN], BF16, tag="g1b")
            nc.scalar.copy(g1b[:, :SN], g1p[:, :SN])
            # sel expand -> multiply v; pairs take one full-width DVE via
            # the shifted lower half of vfull; singles stack into shared
            # T tiles (fold summing both)
            tms = [None] * 13
            for g_i, g in enumerate(SEL_GROUPS):
                wid = 64 * len(g)
                src, bp, csz = SEL_WIN[g_i]
                scol = wcols2["sel"] + g_i * 128
                gt = g0b if src == 0 else g1b
                psp = sps.tile([128, CONVN], F32, tag="s")
                o_up = zS0 + S_OFF[g[0]][0] * PW + S_OFF[g[0]][1]
                if len(g) == 2:
                    nc.tensor.matmul(psp[0:128, :SN],
                                     w2[bp:bp + csz, scol:scol + wid],
                                     gt[bp:bp + csz, :SN],
                                     start=True, stop=True)
                    t_i = g_i if g_i < 2 else g_i - sum(
                        1 for k in SINGLE_T if k < g_i)
                    tm = tpool.tile([128, CONVN], BF16, tag=f"T{t_i}")
                    nc.vector.tensor_tensor(tm[:, :SN], psp[:, :SN],
                                            vfull[:, o_up:o_up + SN],
                                            OP.mult)
                    tms[t_i] = (tm, 2)
                else:
                    t_i, half = SINGLE_T[g_i]
                    if half == 0:
                        tm = tpool.tile([128, CONVN], BF16, tag=f"T{t_i}",
                                        name=f"tms{t_i}")
                    else:
                        tm = tms[t_i][0]
                    if half == 0:
                        nc.tensor.matmul(psp[0:64, :SN],
                                         w2[bp:bp + csz, scol:scol + 64],
                                         gt[bp:bp + csz, :SN],
                                         start=True, stop=True)
                        nc.vector.tensor_tensor(
                            tm[0:64, :SN], psp[0:64, :SN],
                            vfull[0:64, o_up:o_up + SN], OP.mult)
                        tms[t_i] = (tm, 1)
                    else:
                        nc.tensor.matmul(psp[64:128, :SN],
                                         w2[bp:bp + csz, scol:scol + 64],
                                         gt[bp:bp + csz, :SN],
                                         start=True, stop=True)
                        nc.vector.tensor_tensor(
                            tm[64:128, :SN], psp[64:128, :SN],
                            vfull[64:128, o_up - 1:o_up - 1 + SN], OP.mult)
                        tms[t_i] = (tm, 2)
            z_state[ci] = tms

        def z_part2(ci):
            t0, t1 = chunk_rows(ci)
            zr0, zr1 = max(t0, ZR0), min(t1, ZR1)
            if zr0 >= zr1:
                return
            zS0 = zr0 * PW + 1
            SN = (zr1 - zr0) * PW - 2
            tms = z_state.pop(ci)
            psy = vy.tile([64, CONVN], F32, tag="y")
            for t_i, (tm, nh) in enumerate(tms):
                if nh == 2:
                    nc.tensor.matmul(psy[:, :SN], fold128[:, :], tm[:, :SN],
                                     start=(t_i == 0), stop=(t_i == 12))
                else:
                    nc.tensor.matmul(psy[:, :SN], wb[0:64, 64:128],
                                     tm[0:64, :SN],
                                     start=(t_i == 0), stop=(t_i == 12))
            yb = ypool.tile([64, CONVN], BF16, tag="yb")
            nc.scalar.copy(yb[:, :SN], psy[:, :SN])

            def proj(psz, wcol, bias_name):
                nc.tensor.matmul(psz[:, :SN], wb[0:64, wcol:wcol + 64],
                                 yb[:, :SN], start=True, stop=False)
                nc.tensor.matmul(psz[:, :SN], wr_at(wr, bias_name, 64, 66),
                                 x2[64:66, zS0:zS0 + SN],
                                 start=False, stop=True)

            psz = vy.tile([64, CONVN], F32, tag="z")
            proj(psz, 0, "bpi")
            # rows [ZR0, 7) and [71, ZR1) may be out-of-image (which side
            # depends on the half); clamp those z pixels to 0 via
            # relu(psz - L*invind) - relu(-psz - L*invind).
            edge = [(r0, r1) for (r0, r1) in ((zr0, min(zr1, 7)),
                                              (max(zr0, 71), zr1))
                    if r0 < r1]
            if edge:
                pszn = mps.tile([64, CONVN], F32, tag="m")
                proj(pszn, 128, "bpineg")
                for (r0, r1) in edge:
                    e0 = r0 * PW + 1 - zS0
                    en = (r1 - r0) * PW - 2
                    tp = tpool.tile([64, CONVN], F32, tag="zp", bufs=1)
                    nc.scalar.activation(tp[:, :en], psz[:, e0:e0 + en],
                                         AF.Relu)
                    tn = tpool.tile([64, CONVN], F32, tag="zn", bufs=1)
                    nc.scalar.activation(tn[:, :en], pszn[:, e0:e0 + en],
                                         AF.Relu)
                    nc.vector.tensor_tensor(
                        z[0:64, r0 * PW + 1:r0 * PW + 1 + en],
                        tp[:, :en], tn[:, :en], OP.subtract)
            m0, m1 = max(zr0, 7), min(zr1, 71)
            if m0 < m1:
                e0 = m0 * PW + 1 - zS0
                en = (m1 - m0) * PW - 2
                nc.scalar.copy(z[0:64, m0 * PW + 1:m0 * PW + 1 + en],
                               psz[:, e0:e0 + en])
            # mirror the written rows into the stacked lower half
            nc.sync.dma_start(z[64:128, zS0 + PW:zS0 + PW + SN],
                              z[0:64, zS0:zS0 + SN])

        # pipeline: head(ci) fills the PE while part2(ci-2)/part1(ci-1)
        # cover the serial exp/ln waits of chunk ci's A stage.
        for ci in range(n_chunks):
            E = a_head(ci)
            if ci >= 2:
                z_part2(ci - 2)
            a_tail(ci, E)
            if ci >= 1:
                z_part1(ci - 1)
        z_part2(n_chunks - 2)
        z_part1(n_chunks - 1)
        z_part2(n_chunks - 1)
        z3 = z[0:64].bitcast(U16).rearrange("c (r w) -> c r w", r=TROWS, w=PW)
        nc.gpsimd.memset(z3[:, ZR0:ZR1, 0:1], 0.0)
        nc.gpsimd.memset(z3[:, ZR0:ZR1, PW - 1:PW], 0.0)
        z3l = z[64:128].bitcast(U16).rearrange("c (r w) -> c r w",
                                               r=TROWS, w=PW)
        nc.gpsimd.memset(z3l[:, ZR0 + 1:ZR1 + 1, 0:1], 0.0)
        nc.gpsimd.memset(z3l[:, ZR0 + 1:ZR1 + 1, PW - 1:PW], 0.0)

        if DEBUG_DUMP:
            dpool = tc.alloc_tile_pool(name="dbgp", bufs=1)
            NH = NPX // 2
            dt_ = dpool.tile([128, NH], F32, tag="d")
            for j in range(2):
                nc.vector.tensor_copy(dt_[:], vfull[:, j * NH:(j + 1) * NH])
                nc.sync.dma_start(dbg[:, j * NH:(j + 1) * NH], dt_[:])
            for j in range(2):
                nc.vector.tensor_copy(dt_[0:64, :],
                                      z[0:64, j * NH:(j + 1) * NH])
                nc.sync.dma_start(dbg[0:64, NPX + j * NH:NPX + (j + 1) * NH],
                                  dt_[0:64, :])
            for i in range(2):
                at = a_tiles[(23, i)]
                nc.vector.tensor_copy(dt_[:, 0:5 * PW], at[:])
                nc.sync.dma_start(
                    dbg[:, 3 * NPX + i * 5 * PW:3 * NPX + (i + 1) * 5 * PW],
                    dt_[:, 0:5 * PW])
            dpool.release()

        for pool in (gps, sps, vy, mps, ypool, gpool_s, tpool, rpool,
                     epool, apool):
            pool.release()

        # ---- convs 3,4 and output ----
        y3s = spool.tile([128, NPX], BF16, tag="sbig")
        yepool = tc.alloc_tile_pool(name="ye", bufs=2)
        with tc.tile_pool(name="cps2", bufs=4, space="PSUM") as cps2:
            conv_stage(cps2, z, y3s, 2, 6, 72, stack=True)
            # conv4: 3-image-row windows evacuated straight to the output
            y3d = y[:].rearrange("c (r w) -> c r w", r=64, w=W)
            for r in range(7, 71, 3):
                nr = min(3, 71 - r)
                o = r * PW + 1
                n = nr * PW - 2
                ps = cps2.tile([64, CVN], F32)
                conv_matmuls(ps, y3s, 3, o, n)
                yb = yepool.tile([64, 390], F32, tag="yb")
                nc.scalar.activation(yb[:, :n], ps[:, :n], AF.Relu)
                ybv = AP(yb.tensor, yb.offset,
                         [[390, 64], [PW, nr], [1, W]])
                nc.sync.dma_start(y3d[:, r - 7:r - 7 + nr, :], ybv)
        yepool.release()
        spool.release()
        cpool.release()
    return nc


_CACHE = {}
TRACE_DIR = None
LAST_EXEC_NS = None
DEBUG_DUMP = False


def kernel(**inputs):
    x = np.asarray(inputs["x"], np.float32)
    conv_w = np.asarray(inputs["conv_w"], np.float32)
    args = (conv_w, np.asarray(inputs["bn_g"], np.float32),
            np.asarray(inputs["bn_b"], np.float32),
            np.asarray(inputs["bn_m"], np.float32),
            np.asarray(inputs["bn_v"], np.float32),
            np.asarray(inputs["w_v"], np.float32),
            np.asarray(inputs["b_v"], np.float32),
            np.asarray(inputs["w_attn"], np.float32),
            np.asarray(inputs["b_attn"], np.float32),
            np.asarray(inputs["w_proj"], np.float32),
            np.asarray(inputs["b_proj"], np.float32))
    wts, wcols, wts2, wcols2 = _prep_weights(*args)

    # per-core input shards with indicator channels
    shards = []
    for core in range(N_CORES):
        bb, half = divmod(core, 2)
        s = half * 64
        rowbase = s - 7
        sh = np.zeros((66, TROWS, PW), np.float32)
        r0, r1 = max(0, rowbase + 1), min(H, rowbase + 77)
        sh[:64, r0 - rowbase:r1 - rowbase, 1:1 + W] = x[bb, :, r0:r1, :]
        sh[64, r0 - rowbase:r1 - rowbase, 1:1 + W] = 1.0
        sh[65] = 1.0 - sh[64]
        shards.append(sh.reshape(66, NPX))

    key = "k1" + ("d" if DEBUG_DUMP else "")
    if key not in _CACHE:
        _CACHE[key] = _build(wcols, wts.shape[1], wcols2, wts2.shape[1])
    nc = _CACHE[key]
    in_maps = [{"xs": shards[i], "wts": wts, "wts2": wts2}
               for i in range(N_CORES)]
    kw = {}
    if TRACE_DIR is not None:
        kw = dict(trace=True, tmpdir=TRACE_DIR)
    res = run_bass_kernel_spmd(nc, in_maps, core_ids=list(range(N_CORES)), **kw)
    global LAST_EXEC_NS
    LAST_EXEC_NS = res.exec_time_ns
    if DEBUG_DUMP:
        global LAST_DBG
        LAST_DBG = [res.results[i]["dbg"] for i in range(N_CORES)]
    return out_from(res)


def out_from(res):
    out = np.zeros((B, C, H, W), np.float32)
    for core in range(N_CORES):
        bb, half = divmod(core, 2)
        s = half * 64
        out[bb, :, s:s + 64, :] = res.results[core]["y"].reshape(C, 64, W)
    return out


# revision 29
# speedup vs baseline: 1.7768x; 1.2608x over previous
"""ContrastAwareAttentionBlock Trainium2 Bass kernel (s-space restructure).

Sharding: 8 cores = (batch 4) x (image half: rows 0-63 / 64-127); each core
computes its half with a 6-row halo of redundant compute — no collectives.

Attention is computed in "s-space": with s = off_q - off_p (25 values in a
5x5 grid), fold+proj collapse to  z[:,l] = w_proj @ sum_s G_s[n,l] *
v[:, l+s] + b_proj  where  G_s[n,l] = sum_p A[n, l-off_p, p, off_p+s]  and
A is the normalized attention field (packed (p,q,n) channels, 6 tiles of
128).  Per 3-row chunk this needs only 18 G-build matmuls (0/1 selectors
over row-shifted A tiles, accumulated in 2 PSUM tiles of 200 (s,n)
channels), 13 sel expansions ((s,n) -> (s,n,d)), 26 half-DVE multiplies
against a persistent mirrored v field, 13 fold matmuls and a single local
projection — vs 45 sel + 45 fold + 10 shifted-proj matmuls in the direct
(p,q) form.  Invalid pixels die via indicator channels: logits get -BIG,
the softmax denominator gets a huge sentinel channel (exp(ln BIGD) on the
inverse indicator), so A ~ 0 outside the image and all shifted reads are
harmless.  Convs are unchanged: 3x3 conv = 9 PSUM-accumulated matmuls over
shifted slices with BN folded into weights and an indicator-channel bias.
"""
import sys
sys.path.insert(0, '/opt/trn_rl_repo')
import numpy as np

import concourse.bass as bass
import concourse.tile as tile
from concourse import mybir
from concourse.ap import AP
from concourse.bass_utils import run_bass_kernel_spmd

F32 = mybir.dt.float32
BF16 = mybir.dt.bfloat16
U16 = mybir.dt.uint16
AF = mybir.ActivationFunctionType
OP = mybir.AluOpType

B, C, H, W, HEADS, HD = 4, 64, 128, 128, 8, 8
PW = W + 2
TROWS = 78            # tile rows map to image rows [s-7, s+71)
NPX = TROWS * PW
N_CORES = 8
BN_EPS = 1e-5
BIG = 80.0            # exp(-BIG) ~ 0 for invalid pixels
BIGD = 1e30           # denominator for invalid pixels
LARGE = 1e30          # relu clamp for invalid pixels
CONVN = 390           # attention matmul chunk (3 rows, 1 PSUM bank)
CVN = 512             # conv matmul chunk (full PSUM bank)
MAX_WAITS = 1
sc = HD ** -0.5

TAPS = [(di, dj) for di in (-1, 0, 1) for dj in (-1, 0, 1)]

# packed-E layout: 6 tiles x 128; p0..p5 whole, p6,p7,p8 split
P_SEGS = {
    0: [(0, 0, 0, 72)],
    1: [(1, 16, 0, 72)],
    2: [(2, 32, 0, 72)],
    3: [(3, 48, 0, 72)],
    4: [(4, 0, 0, 72)],
    5: [(5, 0, 0, 72)],
    6: [(0, 72, 0, 56), (1, 0, 56, 72)],
    7: [(1, 88, 0, 40), (2, 0, 40, 72)],
    8: [(2, 104, 0, 24), (3, 0, 24, 72)],
}


def _packed_pos(p, q, n):
    ch = 8 * q + n
    for (t, r0, lo, hi) in P_SEGS[p]:
        if lo <= ch < hi:
            return t, r0 + (ch - lo)
    raise AssertionError


def _s_idx(p, q):
    si = TAPS[q][0] - TAPS[p][0]
    sj = TAPS[q][1] - TAPS[p][1]
    return (si + 2) * 5 + (sj + 2)


S_OFF = [((i // 5) - 2, (i % 5) - 2) for i in range(25)]
# pairs are horizontally adjacent s values (one full-width DVE multiply
# against [v; v<<1col]); the per-s-row leftovers are singles stacked into
# shared T tiles (ST0: 4+9, ST1: 14+15, ST2: 24).
SEL_GROUPS = [(0, 1), (2, 3), (4,), (5, 6), (7,), (8, 9), (10, 11),
              (12, 13), (14,), (15,), (16, 17), (18, 19), (20, 21),
              (22, 23), (24,)]
# sel matmul windows: (src 0=G0b/1=G1b, base partition, size); matmul
# partition windows must start at 0/32/64 and stay inside the quadrant
# (base 32: <=32 rows, base 64: <=64 rows); lhsT base must match rhs.
SEL_WIN = [(0, 0, 32), (0, 0, 32), (0, 32, 8), (0, 32, 32), (0, 32, 32),
           (0, 64, 32), (0, 64, 32), (0, 64, 64), (0, 64, 64), (0, 64, 64),
           (1, 0, 32), (1, 0, 32), (1, 32, 32), (1, 32, 32), (1, 64, 8)]
# single groups: (group idx, T-tile id, half 0=up/1=low)
SINGLE_T = {2: (10, 0), 4: (10, 1), 8: (11, 0), 9: (11, 1), 14: (12, 0)}
# G-build segments: (p, tile, has_g0, has_g1); built to match P_SEGS
GB_SEGS = []
for _p in range(9):
    for (_t, _r0, _lo, _hi) in P_SEGS[_p]:
        _g0 = any(8 * _s_idx(_p, ch // 8) + ch % 8 < 128
                  for ch in range(_lo, _hi))
        _g1 = any(8 * _s_idx(_p, ch // 8) + ch % 8 >= 128
                  for ch in range(_lo, _hi))
        GB_SEGS.append((_p, _t, _g0, _g1))


def _split_excess_waits(nc):
    """This walrus build rejects >1 sync wait per instruction; move excess
    waits onto same-engine NOPs inserted before the offender."""
    bbs, fixups = [], {}
    for f in nc.m.functions:
        for bb in f.blocks:
            bbs.append(bb)
            for inst in bb.instructions:
                si = inst.sync_info
                waits = list(si.on_wait) if si is not None and si.on_wait else []
                if len(waits) > MAX_WAITS:
                    si.on_wait = waits[:MAX_WAITS]
                    rest = waits[MAX_WAITS:]
                    chunks = [rest[i:i + MAX_WAITS]
                              for i in range(0, len(rest), MAX_WAITS)]
                    fixups.setdefault(id(bb), {}).setdefault(inst.name, []).extend(
                        (inst.engine, ch) for ch in chunks)
    if not fixups:
        return
    created = {}
    for bb in bbs:
        for name, specs in fixups.get(id(bb), {}).items():
            nops = []
            for engine, ch in specs:
                bi = nc.engines[engine].nop(nofuse=True)
                bi.ins.sync_info = mybir.SyncInfo(on_wait=ch, on_update=[])
                nops.append(bi.ins)
            created[name] = nops
    all_nops = {n.name for ns in created.values() for n in ns}
    for bb in bbs:
        insts = [i for i in bb.instructions if i.name not in all_nops]
        new = []
        for inst in insts:
            new.extend(created.get(inst.name, ()))
            new.append(inst)
        bb.instructions = new


class _TCtx(tile.TileContext):
    def _drain_and_barrier(self, tick_clock, wait_clock):
        from concourse.tile import ScopedClock
        probe = self.nc.sync.nop(nofuse=True)
        wait_clock.add_sem_waits(
            probe.ins, ScopedClock({None: tick_clock.global_clock}))
        self.nc.sync.drain()
        self.nc.all_engine_barrier()
        assert self.sems is not None
        popped = self.nc._tile_sem_poison_stack.pop()
        assert popped is self._sem_poison
        self.nc.clear_and_free_semaphores(list(self.sems.allocated().values()))
        self.nc.all_engine_barrier()

    def __exit__(self, exc_type, exc_val, exc_tb):
        ret = super().__exit__(exc_type, exc_val, exc_tb)
        if exc_type is None:
            _split_excess_waits(self.nc)
        return ret


def _prep_weights(conv_w, bn_g, bn_b, bn_m, bn_v, w_v, b_v, w_attn, b_attn,
                  w_proj, b_proj):
    inv = bn_g / np.sqrt(bn_v + BN_EPS)          # [4, 64]
    beta = bn_b - bn_m * inv                     # [4, 64]
    cols, parts = {}, []

    def add(name, a):
        a = np.asarray(a, np.float32)
        full = np.zeros((72, a.shape[1]), np.float32)
        full[:a.shape[0]] = a
        cols[name] = sum(p.shape[1] for p in parts)
        parts.append(full)

    # conv taps: pairs {(0,dj),(-1,dj)} contract 128 over the row-stacked
    # input ([x; x shifted one row down]); singles (1,dj) contract 64;
    # beta + the -LARGE out-of-image clamp ride a 2-row indicator matmul.
    def wtap(k, di, dj):
        return conv_w[k, :, :, di + 1, dj + 1].T * inv[k][None, :]

    csing = np.zeros((64, 4 * 3 * 64), np.float32)
    cbias = np.zeros((2, 4 * 64), np.float32)
    for k in range(4):
        for j, dj in enumerate((-1, 0, 1)):
            csing[:, (k * 3 + j) * 64:(k * 3 + j) * 64 + 64] = wtap(k, 1, dj)
        cbias[0, k * 64:k * 64 + 64] = beta[k]
        cbias[1, k * 64:k * 64 + 64] = -LARGE
    add("csing", csing)
    add("cbias", cbias)
    wv65 = np.zeros((65, 64), np.float32)
    wv65[:64] = w_v.T
    wv65[64] = b_v
    add("wv", wv65)
    # packed attention logits [66, 6*128]: channel (p,q,n) at _packed_pos;
    # bias on indicator row, -BIG on inverse-indicator row.
    wa = np.zeros((66, 6 * 128), np.float32)
    for p in range(9):
        for q in range(9):
            for n in range(HEADS):
                row = n * 81 + p * 9 + q
                t, r = _packed_pos(p, q, n)
                wa[:64, t * 128 + r] = w_attn[row] * sc
                wa[64, t * 128 + r] = b_attn[row] * sc
                wa[65, t * 128 + r] = -BIG
    add("wa", wa)
    bigd72 = np.zeros((66, 72), np.float32)
    bigd72[65] = BIGD                            # inverse-indicator row
    add("bigd72", bigd72)
    # bct: per tile, [72 -> 128] replicating compact rden (p,n) into the
    # packed layout (zero for pad channels).
    bct = np.zeros((72, 6 * 128), np.float32)
    for p in range(9):
        for q in range(9):
            for n in range(HEADS):
                t, r = _packed_pos(p, q, n)
                bct[8 * p + n, t * 128 + r] = 1.0
    add("bct", bct)
    idm = np.zeros((72, 64), np.float32)
    idm[:64] = np.eye(64, dtype=np.float32)
    add("id64", idm)
    add("wp", w_proj.T)
    bpi = np.zeros((66, 64), np.float32)
    bpi[64] = b_proj                             # indicator row
    bpi[65] = -LARGE
    add("bpi", bpi)
    bpineg = np.zeros((66, 64), np.float32)
    bpineg[64] = -b_proj
    bpineg[65] = -LARGE
    add("bpineg", bpineg)

    # ---- 128-partition weights (wts2) ----
    cols2, parts2 = {}, []

    def add2(name, a):
        a = np.asarray(a, np.float32)
        full = np.zeros((128, a.shape[1]), np.float32)
        full[:a.shape[0]] = a
        cols2[name] = sum(p.shape[1] for p in parts2)
        parts2.append(full)

    # fold128: [128, 64], sums upper+lower 64-halves
    fold = np.zeros((128, 64), np.float32)
    fold[:64] = np.eye(64, dtype=np.float32)
    fold[64:] = np.eye(64, dtype=np.float32)
    add2("fold", fold)
    cpair = np.zeros((128, 4 * 3 * 64), np.float32)
    for k in range(4):
        for j, dj in enumerate((-1, 0, 1)):
            cpair[0:64, (k * 3 + j) * 64:(k * 3 + j) * 64 + 64] = \
                wtap(k, 0, dj)
            cpair[64:128, (k * 3 + j) * 64:(k * 3 + j) * 64 + 64] = \
                wtap(k, -1, dj)
    add2("cpair", cpair)
    # den6: per tile, [128 -> 72] compact softmax denominator (p,n)
    den6 = np.zeros((128, 6 * 72), np.float32)
    for p in range(9):
        for q in range(9):
            for n in range(HEADS):
                t, r = _packed_pos(p, q, n)
                den6[r, t * 72 + 8 * p + n] = 1.0
    add2("den6", den6)
    # G-build: per (p, seg): [128, 128] -> G0 and/or [128, 72] -> G1
    gb0 = np.zeros((128, 0), np.float32)
    g0blocks, g1blocks = [], []
    for (p, t, has_g0, has_g1) in GB_SEGS:
        w0 = np.zeros((128, 128), np.float32)
        w1 = np.zeros((128, 72), np.float32)
        for (tt, r0, lo, hi) in P_SEGS[p]:
            if tt != t:
                continue
            for ch in range(lo, hi):
                q, n = divmod(ch, 8)
                oc = 8 * _s_idx(p, q) + n
                if oc < 128:
                    w0[r0 + (ch - lo), oc] = 1.0
                else:
                    w1[r0 + (ch - lo), oc - 128] = 1.0
        if has_g0:
            g0blocks.append(w0)
        if has_g1:
            g1blocks.append(w1)
    add2("gb0", np.concatenate(g0blocks, axis=1))
    add2("gb1", np.concatenate(g1blocks, axis=1))
    # sel: per group a [128, 128] block; weight rows sit inside the aligned
    # contract window [base, base+csz) matching the G-tile rows read.
    selw = np.zeros((128, 15 * 128), np.float32)
    for g_i, g in enumerate(SEL_GROUPS):
        src, bp, csz = SEL_WIN[g_i]
        r0 = 8 * g[0] - 128 * src
        for j, si in enumerate(g):
            for n in range(HEADS):
                for dd in range(HD):
                    selw[r0 + 8 * (si - g[0]) + n,
                         g_i * 128 + 64 * j + 8 * n + dd] = 1.0
    add2("sel", selw)
    return (np.concatenate(parts, axis=1), cols,
            np.concatenate(parts2, axis=1), cols2)


def _build(wcols, wtotal, wcols2, wtotal2):
    nc = bass.Bass("TRN2", target_bir_lowering=False, debug=False)
    xs = nc.dram_tensor("xs", [66, NPX], F32, kind="ExternalInput").ap()
    wts = nc.dram_tensor("wts", [72, wtotal], F32, kind="ExternalInput").ap()
    wts2 = nc.dram_tensor("wts2", [128, wtotal2], F32,
                          kind="ExternalInput").ap()
    y = nc.dram_tensor("y", [C, 64 * W], F32, kind="ExternalOutput").ap()
    dbg = None
    if DEBUG_DUMP:
        dbg = nc.dram_tensor("dbg", [128, 3 * NPX + 2 * 5 * PW], F32,
                             kind="ExternalOutput").ap()

    def wr_at(w, name, r0, r1, n=64):
        return w[r0:r1, wcols[name]:wcols[name] + n]

    with _TCtx(nc) as tc:
        cpool = tc.alloc_tile_pool(name="const", bufs=1)
        spool = tc.alloc_tile_pool(name="stage", bufs=2)
        rpool_ = tc.alloc_tile_pool(name="raw", bufs=1, side="right")

        wf = rpool_.tile([72, wtotal], F32, tag="wf")
        nc.sync.dma_start(wf[:], wts[:])
        wr = cpool.tile([72, wtotal], BF16)
        nc.vector.tensor_copy(wr[:], wf[:])
        w2f = rpool_.tile([128, wtotal2], F32, tag="w2f")
        nc.sync.dma_start(w2f[:], wts2[:])
        w2 = cpool.tile([128, wtotal2], BF16)
        nc.vector.tensor_copy(w2[:], w2f[:])
        fold128 = w2[:, wcols2["fold"]:wcols2["fold"] + 64]
        wb = cpool.tile([72, 192], BF16)
        nc.vector.tensor_copy(wb[:, 0:64], wf[:, wcols["wp"]:wcols["wp"] + 64])
        nc.vector.tensor_copy(wb[:, 64:128],
                              wf[:, wcols["id64"]:wcols["id64"] + 64])
        nc.vector.tensor_scalar_mul(wb[:, 128:192],
                                    wf[:, wcols["wp"]:wcols["wp"] + 64], -1.0)

        ind2 = cpool.tile([2, NPX], BF16)

        def conv_matmuls(ps, src, k, o, n):
            first = True
            for j in range(3):
                cp = wcols2["cpair"] + (k * 3 + j) * 64
                nc.tensor.matmul(ps[:, :n], w2[0:128, cp:cp + 64],
                                 src[0:128, o + j - 1:o + j - 1 + n],
                                 start=first, stop=False)
                first = False
            for j in range(3):
                cs = wcols["csing"] + (k * 3 + j) * 64
                nc.tensor.matmul(ps[:, :n], wr[0:64, cs:cs + 64],
                                 src[0:64, o + PW + j - 1:o + PW + j - 1 + n],
                                 start=False, stop=False)
            cb = wcols["cbias"] + k * 64
            nc.tensor.matmul(ps[:, :n], wr[0:2, cb:cb + 64],
                             ind2[0:2, o:o + n], start=False, stop=True)

        def conv_stage(cps, src, dst, k, r0, r1, stack):
            lo, hi = r0 * PW + 1, r1 * PW - 1
            for o in range(lo, hi, CVN):
                n = min(CVN, hi - o)
                ps = cps.tile([64, CVN], F32)
                conv_matmuls(ps, src, k, o, n)
                nc.scalar.activation(dst[0:64, o:o + n], ps[:, :n], AF.Relu)
                if stack:
                    nc.sync.dma_start(dst[64:128, o + PW:o + PW + n],
                                      dst[0:64, o:o + n])
            # interior padding cols come out exactly 0 from the -LARGE
            # indicator clamp + ReLU; only the two corner elements outside
            # the evacuation span [lo, hi) need explicit zeroing.
            du = dst[0:64].bitcast(U16)
            nc.gpsimd.memset(du[:, r0 * PW:r0 * PW + 1], 0.0)
            nc.gpsimd.memset(du[:, r1 * PW - 1:r1 * PW], 0.0)
            if stack:
                dl = dst[64:128].bitcast(U16)
                nc.gpsimd.memset(dl[:, (r0 + 1) * PW:(r0 + 1) * PW + 1], 0.0)
                nc.gpsimd.memset(dl[:, (r1 + 1) * PW - 1:(r1 + 1) * PW], 0.0)

        # ---- input & convs 1,2 ----
        x0f = rpool_.tile([66, NPX], F32, tag="x0f")
        nc.sync.dma_start(x0f[:], xs[:])
        x0s = spool.tile([128, NPX], BF16, tag="sbig")
        nc.vector.tensor_copy(ind2[:], x0f[64:66, :])
        NQ = NPX // 4
        for j in range(4):
            a, b = j * NQ, NPX if j == 3 else (j + 1) * NQ
            nc.vector.tensor_copy(x0s[0:64, a:b], x0f[0:64, a:b])
            nc.sync.dma_start(x0s[64:128, a + PW:b + PW] if j < 3 else
                              x0s[64:128, a + PW:NPX],
                              x0s[0:64, a:b] if j < 3 else
                              x0s[0:64, a:NPX - PW])
        rpool_.release()
        x1s = spool.tile([128, NPX], BF16, tag="sbig")
        x2 = spool.tile([66, NPX], BF16, tag="stage", bufs=1)
        nc.sync.dma_start(x2[64:66, :], ind2[0:2, :])
        with tc.tile_pool(name="cps", bufs=4, space="PSUM") as cps:
            conv_stage(cps, x0s, x1s, 0, 2, 76, stack=True)
            conv_stage(cps, x1s, x2, 1, 3, 75, stack=False)

        # ---- attention (s-space) ----
        z = spool.tile([128, NPX], BF16, tag="sbig")
        vfull = spool.tile([128, NPX], BF16, tag="vf", bufs=1)
        nc.gpsimd.memset(vfull[:].bitcast(U16), 0.0)

        GR0, GR1 = 4, 74
        ZR0, ZR1 = 5, 73
        n_chunks = (GR1 - GR0 + 2) // 3

        apool = tc.alloc_tile_pool(name="ap", bufs=2)
        epool = tc.alloc_tile_pool(name="ep", bufs=2)
        rpool = tc.alloc_tile_pool(name="rd", bufs=2)
        tpool = tc.alloc_tile_pool(name="tmp", bufs=2)
        gpool_s = tc.alloc_tile_pool(name="gb", bufs=2)
        ypool = tc.alloc_tile_pool(name="yb", bufs=2)
        mps = tc.alloc_tile_pool(name="mps", bufs=2, space="PSUM")
        vy = tc.alloc_tile_pool(name="vy", bufs=1, space="PSUM")
        sps = tc.alloc_tile_pool(name="sps", bufs=2, space="PSUM")
        gps = tc.alloc_tile_pool(name="gps", bufs=1, space="PSUM")

        a_tiles = {}

        def chunk_rows(ci):
            t0 = GR0 + 3 * ci
            return t0, min(t0 + 3, GR1)

        z_state = {}

        def a_head(ci):
            t0, t1 = chunk_rows(ci)
            S0 = t0 * PW + 1
            SN = (t1 - t0) * PW - 2
            # v rows [3+3ci, min(6+3ci, 75)) into persistent vfull whose
            # lower half is v shifted one column left (lower[l] = v[l+1])
            rv0, rv1 = 3 + 3 * ci, min(6 + 3 * ci, 75)
            vlo, vn = rv0 * PW + 1, (rv1 - rv0) * PW - 2
            psv = mps.tile([64, CONVN], F32, tag="m")
            nc.tensor.matmul(psv[:, :vn], wr_at(wr, "wv", 0, 65),
                             x2[0:65, vlo:vlo + vn], start=True, stop=True)
            nc.scalar.copy(vfull[0:64, vlo:vlo + vn], psv[:, :vn])
            nc.sync.dma_start(vfull[64:128, vlo - 1:vlo - 1 + vn],
                              vfull[0:64, vlo:vlo + vn])
            # packed logits -> exp
            E = []
            for i in range(6):
                psl = mps.tile([128, CONVN], F32, tag="m")
                nc.tensor.matmul(psl[:, :SN],
                                 wr[0:66, wcols["wa"] + i * 128:
                                    wcols["wa"] + i * 128 + 128],
                                 x2[0:66, S0:S0 + SN], start=True, stop=True)
                e = epool.tile([128, CONVN], BF16, tag=f"E{i}")
                nc.scalar.activation(e[:, :SN], psl[:, :SN], AF.Exp)
                E.append(e)
            return E

        rdcbs = {}

        def a_den(ci, E):
            t0, t1 = chunk_rows(ci)
            SN = (t1 - t0) * PW - 2
            # compact denominator (p,n) + reciprocal straight to bf16 (its
            # consumer is a bf16 matmul); runs a full chunk ahead of bct so
            # the DVE reciprocal never stalls the PE.
            psd = mps.tile([128, CONVN], F32, tag="m")
            for i in range(6):
                nc.tensor.matmul(psd[0:72, :SN],
                                 w2[0:128, wcols2["den6"] + i * 72:
                                    wcols2["den6"] + i * 72 + 72],
                                 E[i][:, :SN], start=(i == 0), stop=False)
            nc.tensor.matmul(psd[0:72, :SN], wr_at(wr, "bigd72", 64, 66, 72),
                             x2[64:66, t0 * PW + 1:t0 * PW + 1 + SN],
                             start=False, stop=True)
            rdcb = rpool.tile([72, CONVN], BF16, tag="rdcb")
            with nc.allow_low_precision("rden is consumed as bf16 weights"):
                nc.vector.reciprocal(rdcb[:, :SN], psd[0:72, :SN])
            rdcbs[ci] = rdcb

        def a_fin(ci, E):
            t0, t1 = chunk_rows(ci)
            SN = (t1 - t0) * PW - 2
            rdcb = rdcbs.pop(ci)
            # A tiles (normalized attention field) with 1-row halo layout:
            # [128, 5PW], own rows at [PW, (1+rows)*PW)
            for i in range(6):
                psr = mps.tile([128, CONVN], F32, tag="m")
                nc.tensor.matmul(psr[:, :SN],
                                 wr[0:72, wcols["bct"] + i * 128:
                                    wcols["bct"] + i * 128 + 128],
                                 rdcb[:, :SN], start=True, stop=True)
                at = apool.tile([128, 5 * PW], BF16, tag=f"A{i}")
                nc.vector.tensor_tensor(at[:, PW + 1:PW + 1 + SN],
                                        psr[:, :SN], E[i][:, :SN], OP.mult)
                atu = at[:].bitcast(U16)
                if ci == 0:
                    nc.gpsimd.memset(atu[:, 0:PW + 1], 0.0)
                else:
                    nc.gpsimd.memset(atu[:, PW:PW + 1], 0.0)
                nc.gpsimd.memset(atu[:, PW + 1 + SN:PW + 2 + SN], 0.0)
                a_tiles[(ci, i)] = at
            # halo exchange with previous chunk (first/last own rows)
            if ci > 0:
                for i in range(6):
                    prev = a_tiles[(ci - 1, i)]
                    cur = a_tiles[(ci, i)]
                    nc.sync.dma_start(cur[:, 0:PW], prev[:, 3 * PW:4 * PW])
                    nc.sync.dma_start(prev[:, 4 * PW:5 * PW],
                                      cur[:, PW:2 * PW])
                    a_tiles.pop((ci - 3, i), None)

        def z_part1(ci):
            t0, t1 = chunk_rows(ci)
            zr0, zr1 = max(t0, ZR0), min(t1, ZR1)
            if zr0 >= zr1:
                return
            zS0 = zr0 * PW + 1
            SN = (zr1 - zr0) * PW - 2
            base = (t0 - 1) * PW
            # G-build: 18 shifted 0/1 matmuls accumulating (s,n) channels
            g0p = gps.tile([128, CONVN], F32, tag="g0")
            g1p = gps.tile([72, CONVN], F32, tag="g1")
            n0 = n1 = 0
            i0 = sum(1 for (_, _, h0, _) in GB_SEGS if h0)
            i1 = sum(1 for (_, _, _, h1) in GB_SEGS if h1)
            for (p, t, has_g0, has_g1) in GB_SEGS:
                di, dj = TAPS[p]
                off = di * PW + dj
                la = zS0 - off - base
                at = a_tiles[(ci, t)]
                if has_g0:
                    c0 = wcols2["gb0"] + n0 * 128
                    nc.tensor.matmul(g0p[:, :SN], w2[0:128, c0:c0 + 128],
                                     at[:, la:la + SN],
                                     start=(n0 == 0), stop=(n0 == i0 - 1))
                    n0 += 1
                if has_g1:
                    c1 = wcols2["gb1"] + n1 * 72
                    nc.tensor.matmul(g1p[:, :SN], w2[0:128, c1:c1 + 72],
                                     at[:, la:la + SN],
                                     start=(n1 == 0), stop=(n1 == i1 - 1))
                    n1 += 1
            g0b = gpool_s.tile([128, CONVN], BF16, tag="g0b")
            nc.scalar.copy(g0b[:, :SN], g0p[:, :SN])
            g1b = gpool_s.tile([72, CONVN], BF16, tag="g1b")
            nc.scalar.copy(g1b[:, :SN], g1p[:, :SN])
            # sel expand -> multiply v; pairs take one full-width DVE via
            # the shifted lower half of vfull; singles stack into shared
            # T tiles (fold summing both)
            tms = [None] * 13
            for g_i, g in enumerate(SEL_GROUPS):
                wid = 64 * len(g)
                src, bp, csz = SEL_WIN[g_i]
                scol = wcols2["sel"] + g_i * 128
                gt = g0b if src == 0 else g1b
                psp = sps.tile([128, CONVN], F32, tag="s")
                o_up = zS0 + S_OFF[g[0]][0] * PW + S_OFF[g[0]][1]
                if len(g) == 2:
                    nc.tensor.matmul(psp[0:128, :SN],
                                     w2[bp:bp + csz, scol:scol + wid],
                                     gt[bp:bp + csz, :SN],
                                     start=True, stop=True)
                    t_i = g_i if g_i < 2 else g_i - sum(
                        1 for k in SINGLE_T if k < g_i)
                    tm = tpool.tile([128, CONVN], BF16, tag=f"T{t_i}")
                    nc.vector.tensor_tensor(tm[:, :SN], psp[:, :SN],
                                            vfull[:, o_up:o_up + SN],
                                            OP.mult)
                    tms[t_i] = (tm, 2)
                else:
                    t_i, half = SINGLE_T[g_i]
                    if half == 0:
                        tm = tpool.tile([128, CONVN], BF16, tag=f"T{t_i}",
                                        name=f"tms{t_i}")
                    else:
                        tm = tms[t_i][0]
                    if half == 0:
                        nc.tensor.matmul(psp[0:64, :SN],
                                         w2[bp:bp + csz, scol:scol + 64],
                                         gt[bp:bp + csz, :SN],
                                         start=True, stop=True)
                        nc.vector.tensor_tensor(
                            tm[0:64, :SN], psp[0:64, :SN],
                            vfull[0:64, o_up:o_up + SN], OP.mult)
                        tms[t_i] = (tm, 1)
                    else:
                        nc.tensor.matmul(psp[64:128, :SN],
                                         w2[bp:bp + csz, scol:scol + 64],
                                         gt[bp:bp + csz, :SN],
                                         start=True, stop=True)
                        nc.vector.tensor_tensor(
                            tm[64:128, :SN], psp[64:128, :SN],
                            vfull[64:128, o_up - 1:o_up - 1 + SN], OP.mult)
                        tms[t_i] = (tm, 2)
            z_state[ci] = tms

        def z_part2(ci):
            t0, t1 = chunk_rows(ci)
            zr0, zr1 = max(t0, ZR0), min(t1, ZR1)
            if zr0 >= zr1:
                return
            zS0 = zr0 * PW + 1
            SN = (zr1 - zr0) * PW - 2
            tms = z_state.pop(ci)
            psy = vy.tile([64, CONVN], F32, tag="y")
            for t_i, (tm, nh) in enumerate(tms):
                if nh == 2:
                    nc.tensor.matmul(psy[:, :SN], fold128[:, :], tm[:, :SN],
                                     start=(t_i == 0), stop=(t_i == 12))
                else:
                    nc.tensor.matmul(psy[:, :SN], wb[0:64, 64:128],
                                     tm[0:64, :SN],
                                     start=(t_i == 0), stop=(t_i == 12))
            yb = ypool.tile([64, CONVN], BF16, tag="yb")
            nc.scalar.copy(yb[:, :SN], psy[:, :SN])

            def proj(psz, wcol, bias_name):
                nc.tensor.matmul(psz[:, :SN], wb[0:64, wcol:wcol + 64],
                                 yb[:, :SN], start=True, stop=False)
                nc.tensor.matmul(psz[:, :SN], wr_at(wr, bias_name, 64, 66),
                                 x2[64:66, zS0:zS0 + SN],
                                 start=False, stop=True)

            psz = vy.tile([64, CONVN], F32, tag="z")
            proj(psz, 0, "bpi")
            # rows [ZR0, 7) and [71, ZR1) may be out-of-image (which side
            # depends on the half); clamp those z pixels to 0 via
            # relu(psz - L*invind) - relu(-psz - L*invind).
            edge = [(r0, r1) for (r0, r1) in ((zr0, min(zr1, 7)),
                                              (max(zr0, 71), zr1))
                    if r0 < r1]
            if edge:
                pszn = mps.tile([64, CONVN], F32, tag="m")
                proj(pszn, 128, "bpineg")
                for (r0, r1) in edge:
                    e0 = r0 * PW + 1 - zS0
                    en = (r1 - r0) * PW - 2
                    tp = tpool.tile([64, CONVN], F32, tag="zp", bufs=1)
                    nc.scalar.activation(tp[:, :en], psz[:, e0:e0 + en],
                                         AF.Relu)
                    tn = tpool.tile([64, CONVN], F32, tag="zn", bufs=1)
                    nc.scalar.activation(tn[:, :en], pszn[:, e0:e0 + en],
                                         AF.Relu)
                    nc.vector.tensor_tensor(
                        z[0:64, r0 * PW + 1:r0 * PW + 1 + en],
                        tp[:, :en], tn[:, :en], OP.subtract)
            m0, m1 = max(zr0, 7), min(zr1, 71)
            if m0 < m1:
                e0 = m0 * PW + 1 - zS0
                en = (m1 - m0) * PW - 2
                nc.scalar.copy(z[0:64, m0 * PW + 1:m0 * PW + 1 + en],
                               psz[:, e0:e0 + en])
            # mirror the written rows into the stacked lower half
            nc.sync.dma_start(z[64:128, zS0 + PW:zS0 + PW + SN],
                              z[0:64, zS0:zS0 + SN])

        # depth-3 pipeline: chunk ci's den/reciprocal run one iteration
        # before bct consumes it, and z runs two/three iterations behind,
        # so the PE never waits on the serial exp -> den -> recip chain.
        E_cur = {}
        for it in range(n_chunks + 3):
            if it < n_chunks:
                E_cur[it] = a_head(it)
            if 0 <= it - 1 < n_chunks:
                a_fin(it - 1, E_cur.pop(it - 1))
            if it < n_chunks:
                a_den(it, E_cur[it])
            if 0 <= it - 2 < n_chunks:
                z_part1(it - 2)
            if 0 <= it - 3 < n_chunks:
                z_part2(it - 3)
        z3 = z[0:64].bitcast(U16).rearrange("c (r w) -> c r w", r=TROWS, w=PW)
        nc.gpsimd.memset(z3[:, ZR0:ZR1, 0:1], 0.0)
        nc.gpsimd.memset(z3[:, ZR0:ZR1, PW - 1:PW], 0.0)
        z3l = z[64:128].bitcast(U16).rearrange("c (r w) -> c r w",
                                               r=TROWS, w=PW)
        nc.gpsimd.memset(z3l[:, ZR0 + 1:ZR1 + 1, 0:1], 0.0)
        nc.gpsimd.memset(z3l[:, ZR0 + 1:ZR1 + 1, PW - 1:PW], 0.0)

        if DEBUG_DUMP:
            dpool = tc.alloc_tile_pool(name="dbgp", bufs=1)
            NH = NPX // 2
            dt_ = dpool.tile([128, NH], F32, tag="d")
            for j in range(2):
                nc.vector.tensor_copy(dt_[:], vfull[:, j * NH:(j + 1) * NH])
                nc.sync.dma_start(dbg[:, j * NH:(j + 1) * NH], dt_[:])
            for j in range(2):
                nc.vector.tensor_copy(dt_[0:64, :],
                                      z[0:64, j * NH:(j + 1) * NH])
                nc.sync.dma_start(dbg[0:64, NPX + j * NH:NPX + (j + 1) * NH],
                                  dt_[0:64, :])
            for i in range(2):
                at = a_tiles[(23, i)]
                nc.vector.tensor_copy(dt_[:, 0:5 * PW], at[:])
                nc.sync.dma_start(
                    dbg[:, 3 * NPX + i * 5 * PW:3 * NPX + (i + 1) * 5 * PW],
                    dt_[:, 0:5 * PW])
            dpool.release()

        for pool in (gps, sps, vy, mps, ypool, gpool_s, tpool, rpool,
                     epool, apool):
            pool.release()

        # ---- convs 3,4 and output ----
        y3s = spool.tile([128, NPX], BF16, tag="sbig")
        yepool = tc.alloc_tile_pool(name="ye", bufs=2)
        with tc.tile_pool(name="cps2", bufs=4, space="PSUM") as cps2:
            conv_stage(cps2, z, y3s, 2, 6, 72, stack=True)
            # conv4: 3-image-row windows evacuated straight to the output
            y3d = y[:].rearrange("c (r w) -> c r w", r=64, w=W)
            for r in range(7, 71, 3):
                nr = min(3, 71 - r)
                o = r * PW + 1
                n = nr * PW - 2
                ps = cps2.tile([64, CVN], F32)
                conv_matmuls(ps, y3s, 3, o, n)
                yb = yepool.tile([64, 390], F32, tag="yb")
                nc.scalar.activation(yb[:, :n], ps[:, :n], AF.Relu)
                ybv = AP(yb.tensor, yb.offset,
                         [[390, 64], [PW, nr], [1, W]])
                nc.sync.dma_start(y3d[:, r - 7:r - 7 + nr, :], ybv)
        yepool.release()
        spool.release()
        cpool.release()
    return nc


_CACHE = {}
TRACE_DIR = None
LAST_EXEC_NS = None
DEBUG_DUMP = False


def kernel(**inputs):
    x = np.asarray(inputs["x"], np.float32)
    conv_w = np.asarray(inputs["conv_w"], np.float32)
    args = (conv_w, np.asarray(inputs["bn_g"], np.float32),
            np.asarray(inputs["bn_b"], np.float32),
            np.asarray(inputs["bn_m"], np.float32),
            np.asarray(inputs["bn_v"], np.float32),
            np.asarray(inputs["w_v"], np.float32),
            np.asarray(inputs["b_v"], np.float32),
            np.asarray(inputs["w_attn"], np.float32),
            np.asarray(inputs["b_attn"], np.float32),
            np.asarray(inputs["w_proj"], np.float32),
            np.asarray(inputs["b_proj"], np.float32))
    wts, wcols, wts2, wcols2 = _prep_weights(*args)

    # per-core input shards with indicator channels
    shards = []
    for core in range(N_CORES):
        bb, half = divmod(core, 2)
        s = half * 64
        rowbase = s - 7
        sh = np.zeros((66, TROWS, PW), np.float32)
        r0, r1 = max(0, rowbase + 1), min(H, rowbase + 77)
        sh[:64, r0 - rowbase:r1 - rowbase, 1:1 + W] = x[bb, :, r0:r1, :]
        sh[64, r0 - rowbase:r1 - rowbase, 1:1 + W] = 1.0
        sh[65] = 1.0 - sh[64]
        shards.append(sh.reshape(66, NPX))

    key = "k1" + ("d" if DEBUG_DUMP else "")
    if key not in _CACHE:
        _CACHE[key] = _build(wcols, wts.shape[1], wcols2, wts2.shape[1])
    nc = _CACHE[key]
    in_maps = [{"xs": shards[i], "wts": wts, "wts2": wts2}
               for i in range(N_CORES)]
    kw = {}
    if TRACE_DIR is not None:
        kw = dict(trace=True, tmpdir=TRACE_DIR)
    res = run_bass_kernel_spmd(nc, in_maps, core_ids=list(range(N_CORES)), **kw)
    global LAST_EXEC_NS
    LAST_EXEC_NS = res.exec_time_ns
    if DEBUG_DUMP:
        global LAST_DBG
        LAST_DBG = [res.results[i]["dbg"] for i in range(N_CORES)]
    return out_from(res)


def out_from(res):
    out = np.zeros((B, C, H, W), np.float32)
    for core in range(N_CORES):
        bb, half = divmod(core, 2)
        s = half * 64
        out[bb, :, s:s + 64, :] = res.results[core]["y"].reshape(C, 64, W)
    return out
